# revision 24
# baseline (speedup 1.0000x reference)
"""AttentionSTAE on 8 Trainium2 cores (Bass/Tile).

Structure (hardcoded shapes: N=64, B=64, T=48, F=10, EMB=16, H=128, E=256):

  Sharded data-parallel over batch B: core i handles batch slice
  b in [8i, 8i+8) -> 512 LSTM rows (r = n*8 + b'), pipelined over t:

    enc LSTM L0 (feature-major, gates [i|f|o|g] in PSUM)
    -> enc LSTM L1 (lagged one step)
    -> 6-layer dense per-node MLP (the GAT collapses to a dense layer for
       every node except global graph rows q<64; see host correction below)
    -> dec LSTM L0/L1 (row-major, lagged) -> sigmoid -> out

  The reference tiles the same [2,256] edge list (ids < 64) B*T times
  without offsets, so only global rows q<64 (batch b=0 -> core 0) receive
  real messages; every other node's GAT output is x @ W.T + b. The exact
  GAT path for the 96 affected rows (q<96 covers every decoder row that
  sees a corrected value) is recomputed on the host in float64 and the two
  affected output rows (n=0,1 at b=0) are overwritten.
"""

import numpy as np

N, B, T, F, EMB, H, E = 64, 64, 48, 10, 16, 128, 256
NCORES = 8
BPC = B // NCORES  # batch per core
R = N * BPC  # 512 LSTM rows per core
NEG = np.float32(0.2)

_CACHE = {}


# ---------------------------------------------------------------- device ---


def _build_module(nsteps=T):
    import concourse.bacc as bacc
    import concourse.tile as tile
    from concourse import mybir

    F32 = mybir.dt.float32
    F32R = mybir.dt.float32r
    SIG = mybir.ActivationFunctionType.Sigmoid
    TANH = mybir.ActivationFunctionType.Tanh
    ADD = mybir.AluOpType.add
    MAX = mybir.AluOpType.max
    import concourse.bass as bass

    nc = bacc.Bacc("TRN2", target_bir_lowering=False, debug=False)

    def din(name, shape):
        return nc.dram_tensor(name, shape, F32, kind="ExternalInput")

    xin = din("xin", [F, nsteps, R])
    embr = din("embr", [EMB + 1, R])  # emb rows + ones row (bias trick)
    w0h = din("w0h", [H, 4 * H])
    w0xe = din("w0xe", [F + EMB + 1, 4 * H])
    w1x = din("w1x", [H, 4 * H])
    w1h = din("w1h", [H, 4 * H])
    b1c = din("b1c", [H, 4])
    mlp_dims = [(H, H), (H, 64), (64, 32), (32, 64), (64, H), (H, H)]
    mws = [din(f"mw{i}", [k, m]) for i, (k, m) in enumerate(mlp_dims)]
    mbs = [din(f"mb{i}", [m, 1]) for i, (k, m) in enumerate(mlp_dims)]
    wd0x = din("wd0x", [H, 40])
    wd0h_blk = din("wd0h_blk", [40, 160])
    wd1xh_blk = din("wd1xh_blk", [80, 160])
    bd0t = din("bd0t", [10, 160])
    bd1t = din("bd1t", [10, 160])
    ones10 = din("ones10", [10, 128])
    ident = din("ident", [H, H])
    zeros = din("zeros", [H, R])
    yout = nc.dram_tensor("yout", [128, nsteps * 40], F32, kind="ExternalOutput")

    def cap(base, col_off, dims, part=None):
        """Custom AP over a tile: base partition dim + free dims (elem units)."""
        b = base[:] if not isinstance(base, bass.AP) else base
        pdim = [list(b.ap[0])]
        if part is not None:
            pdim = [[b.ap[0][0], part]]
        return bass.AP(b.tensor, b.offset + col_off, pdim + [list(d) for d in dims])

    with tile.TileContext(nc) as tc:
        with (
            tc.tile_pool(name="pers", bufs=1) as pers,
            tc.tile_pool(name="s0p", bufs=2) as s0p,
            tc.tile_pool(name="s1p", bufs=2) as s1p,
            tc.tile_pool(name="tcp", bufs=2) as tcp,
            tc.tile_pool(name="uvp", bufs=3) as uvp,
            tc.tile_pool(name="mlpa", bufs=2) as mlpa,
            tc.tile_pool(name="decp", bufs=2) as decp,
            tc.tile_pool(name="gA", bufs=1, space="PSUM") as gA,
            tc.tile_pool(name="gB", bufs=1, space="PSUM") as gB,
            tc.tile_pool(name="gC", bufs=1, space="PSUM") as gC,
            tc.tile_pool(name="gD", bufs=1, space="PSUM") as gD,
        ):
            # ---- persistent tiles ----
            h0 = pers.tile([H, R], F32R)
            h1 = pers.tile([H, R], F32R)
            c01 = pers.tile([H, 2 * R], F32)
            cd = pers.tile([128, 80], F32)
            outbuf = pers.tile([128, 80 + nsteps * 40], F32R)
            hd_stage = pers.tile([128, 80], F32R)  # [hd0(t1-1) | hd1(t2-1)]
            hdT = pers.tile([80, 128], F32R)
            xeA = pers.tile([F + EMB + 1, R], F32R)
            xeB = pers.tile([F + EMB + 1, R], F32R)
            w0h_s = pers.tile([H, 4 * H], F32R)
            w0xe_s = pers.tile([F + EMB + 1, 4 * H], F32R)
            w1x_s = pers.tile([H, 4 * H], F32R)
            w1h_s = pers.tile([H, 4 * H], F32R)
            b1_s = pers.tile([H, 4], F32)
            mw_s = [
                pers.tile([k, m], F32R, name=f"mws{i}")
                for i, (k, m) in enumerate(mlp_dims)
            ]
            mb_s = [
                pers.tile([m, 1], F32, name=f"mbs{i}")
                for i, (k, m) in enumerate(mlp_dims)
            ]
            wd0x_s = pers.tile([H, 40], F32R)
            wd0h_s = pers.tile([40, 160], F32R)
            wd1xh_s = pers.tile([80, 160], F32R)
            bd0t_s = pers.tile([10, 160], F32R)
            bd1t_s = pers.tile([10, 160], F32R)
            ones10_s = pers.tile([10, 128], F32R)
            ident_s = pers.tile([H, H], F32R)

            r32 = lambda ap: ap.bitcast(F32R)
            for dst, src in [
                (w0h_s, w0h), (w0xe_s, w0xe), (w1x_s, w1x), (w1h_s, w1h),
                (wd0x_s, wd0x), (wd0h_s, wd0h_blk), (wd1xh_s, wd1xh_blk),
                (bd0t_s, bd0t), (bd1t_s, bd1t), (ones10_s, ones10),
                (ident_s, ident),
            ]:
                nc.sync.dma_start(dst[:], r32(src[:]))
            nc.sync.dma_start(b1_s[:], b1c[:])
            for i in range(6):
                nc.sync.dma_start(mw_s[i][:], r32(mws[i][:]))
                nc.sync.dma_start(mb_s[i][:], mbs[i][:])
            nc.sync.dma_start(xeA[F : F + EMB + 1, :], r32(embr[:]))
            nc.sync.dma_start(xeB[F : F + EMB + 1, :], r32(embr[:]))

            nc.sync.dma_start(h0[:], r32(zeros[:]))
            nc.sync.dma_start(h1[:], r32(zeros[:]))
            nc.sync.dma_start(hd_stage[:], r32(zeros[:, 0:80]))
            nc.gpsimd.memset(c01[:], 0.0)
            nc.gpsimd.memset(cd[:], 0.0)


            KS = nsteps + 2  # python pipeline iters
            for k in range(KS):
                has_l0 = k < nsteps
                has_l1 = 1 <= k <= nsteps
                has_dec1 = k >= 2
                t0 = k  # enc L0 step
                t1 = k - 1  # enc L1 / MLP / dec L0 step
                t2 = k - 2  # dec L1 step

                # ---------------- encoder layer 0 (t0) ----------------
                if has_l0:
                    xe = xeA if k % 2 == 0 else xeB
                    nc.sync.dma_start(
                        xe[0:F, :], r32(xin[:, t0, :])
                    )
                    p0if = gA.tile([H, 2 * R], F32, tag="gA")
                    p0og = gB.tile([H, 2 * R], F32, tag="gB")
                    for m in range(4):
                        reg = (p0if if m < 2 else p0og)[:, (m % 2) * R : (m % 2) * R + R]
                        nc.tensor.matmul(
                            reg, w0h_s[:, m * H : (m + 1) * H], h0[:],
                            start=True, stop=False,
                        )
                        nc.tensor.matmul(
                            reg, w0xe_s[:, m * H : (m + 1) * H], xe[:],
                            start=False, stop=True,
                        )
                    s0 = s0p.tile([H, 4 * R], F32, tag="s0")
                    nc.scalar.activation(s0[:, 0 : 2 * R], p0if[:], SIG)
                    nc.scalar.activation(s0[:, 2 * R : 3 * R], p0og[:, 0:R], SIG)
                    nc.scalar.activation(s0[:, 3 * R : 4 * R], p0og[:, R : 2 * R], TANH)
                    u0 = uvp.tile([H, R], F32, tag="uv")
                    v0 = uvp.tile([H, R], F32, tag="uv")
                    nc.gpsimd.tensor_mul(u0[:], s0[:, 0:R], s0[:, 3 * R : 4 * R])
                    nc.gpsimd.tensor_mul(v0[:], s0[:, R : 2 * R], c01[:, 0:R])
                    nc.vector.tensor_add(c01[:, 0:R], u0[:], v0[:])

                # ---------------- encoder layer 1 (t1) ----------------
                if has_l1:
                    p1if = gC.tile([H, 2 * R], F32, tag="gC")
                    p1og = gD.tile([H, 2 * R], F32, tag="gD")
                    for m in range(4):
                        reg = (p1if if m < 2 else p1og)[:, (m % 2) * R : (m % 2) * R + R]
                        nc.tensor.matmul(
                            reg, w1h_s[:, m * H : (m + 1) * H], h1[:],
                            start=True, stop=False,
                        )
                        nc.tensor.matmul(
                            reg, w1x_s[:, m * H : (m + 1) * H], h0[:],
                            start=False, stop=True,
                        )
                    s1 = s1p.tile([H, 4 * R], F32, tag="s1")
                    nc.scalar.activation(s1[:, 0:R], p1if[:, 0:R], SIG, bias=b1_s[:, 0:1])
                    nc.scalar.activation(s1[:, R : 2 * R], p1if[:, R : 2 * R], SIG, bias=b1_s[:, 1:2])
                    nc.scalar.activation(s1[:, 2 * R : 3 * R], p1og[:, 0:R], SIG, bias=b1_s[:, 2:3])
                    nc.scalar.activation(s1[:, 3 * R : 4 * R], p1og[:, R : 2 * R], TANH, bias=b1_s[:, 3:4])
                    u1 = uvp.tile([H, R], F32, tag="uv")
                    v1 = uvp.tile([H, R], F32, tag="uv")
                    nc.gpsimd.tensor_mul(u1[:], s1[:, 0:R], s1[:, 3 * R : 4 * R])
                    nc.gpsimd.tensor_mul(v1[:], s1[:, R : 2 * R], c01[:, R : 2 * R])
                    nc.vector.tensor_add(c01[:, R : 2 * R], u1[:], v1[:])

                # tanh(c) merged over both layers where possible
                tc01 = tcp.tile([H, 2 * R], F32, tag="tc")
                if has_l0 and has_l1:
                    nc.scalar.activation(tc01[:], c01[:], TANH)
                elif has_l0:
                    nc.scalar.activation(tc01[:, 0:R], c01[:, 0:R], TANH)
                elif has_l1:
                    nc.scalar.activation(tc01[:, R : 2 * R], c01[:, R : 2 * R], TANH)

                if has_l0:
                    nc.gpsimd.tensor_mul(h0[:], s0[:, 2 * R : 3 * R], tc01[:, 0:R])
                if has_l1:
                    nc.gpsimd.tensor_mul(h1[:], s1[:, 2 * R : 3 * R], tc01[:, R : 2 * R])

                # ---------------- MLP (t1) ----------------
                if has_l1:
                    pm = gA.tile([H, 2 * R], F32, tag="gA")
                    prev = None
                    for i, (kk, mm) in enumerate(mlp_dims):
                        rhs = h1[:] if i == 0 else prev[0:kk, :]
                        reg = pm[0:mm, (i % 2) * R : (i % 2) * R + R]
                        nc.tensor.matmul(reg, mw_s[i][:], rhs, start=True, stop=True)
                        a = mlpa.tile([H, R], F32R, tag=f"a{i % 2}")
                        nc.vector.tensor_scalar(
                            a[0:mm, :], reg, mb_s[i][:, 0:1], 0.0, ADD, MAX
                        )
                        prev = a
                    y_t = prev  # [128, R] f32r tile

                # ---------------- decoder (dec L0 @ t1, dec L1 @ t2) ----------
                if has_l1 or has_dec1:
                    pd = gD.tile([H, 2 * R], F32, tag="gD")
                    # transpose [hd0(t1-1) | hd1(t2-1)] -> hdT
                    if k >= 3:
                        nc.vector.tensor_copy(
                            hd_stage[:, 40:80],
                            outbuf[:, 80 + (k - 3) * 40 : 120 + (k - 3) * 40],
                        )
                    tps = cap(pd, 320, [[1, 128]], part=80).bitcast(F32R)
                    nc.tensor.transpose(tps, hd_stage[:], ident_s[:])
                    nc.vector.tensor_copy(hdT[:], tps)

                if has_l1:  # dec L0 at t1
                    nc.tensor.matmul(
                        pd[:, 0:160], ones10_s[:], bd0t_s[:],
                        start=True, stop=False, skip_group_check=True,
                    )
                    for kk in range(4):
                        nc.tensor.matmul(
                            pd[:, kk * 40 : kk * 40 + 40],
                            y_t[:, kk * 128 : (kk + 1) * 128],
                            wd0x_s[:], start=False, stop=False,
                            skip_group_check=True,
                        )
                    nc.tensor.matmul(
                        pd[:, 0:160], hdT[0:40, :], wd0h_s[:],
                        start=False, stop=True, skip_group_check=True,
                    )
                if has_dec1:  # dec L1 at t2 (x from hd0, h from hd1: one mm)
                    nc.tensor.matmul(
                        pd[:, 160:320], hdT[:], wd1xh_s[:],
                        start=True, stop=False, skip_group_check=True,
                    )
                    nc.tensor.matmul(
                        pd[:, 160:320], ones10_s[:], bd1t_s[:],
                        start=False, stop=True, skip_group_check=True,
                    )

                if has_l1 or has_dec1:
                    lt_off = 0 if has_l1 else 4  # active lt range
                    lt_cnt = (4 if has_l1 else 0) + (4 if has_dec1 else 0)
                    sgd = decp.tile([128, 240], F32, tag="sgd")
                    tgd = decp.tile([128, 80], F32, tag="tgd")
                    po, so = lt_off * 40, lt_off * 30
                    nc.scalar.activation(
                        cap(sgd, so, [[30, lt_cnt], [1, 30]]),
                        cap(pd, po, [[40, lt_cnt], [1, 30]]), SIG,
                    )
                    nc.scalar.activation(
                        cap(tgd, lt_off * 10, [[10, lt_cnt], [1, 10]]),
                        cap(pd, po + 30, [[40, lt_cnt], [1, 10]]), TANH,
                    )
                    ud = decp.tile([128, 80], F32, tag="ud")
                    vd = decp.tile([128, 80], F32, tag="vd")
                    co = lt_off * 10
                    cw = lt_cnt * 10
                    i_ap = cap(sgd, so, [[30, lt_cnt], [1, 10]])
                    f_ap = cap(sgd, so + 10, [[30, lt_cnt], [1, 10]])
                    o_ap = cap(sgd, so + 20, [[30, lt_cnt], [1, 10]])
                    g_ap = cap(tgd, co, [[10, lt_cnt], [1, 10]])
                    nc.vector.tensor_mul(ud[:, co : co + cw], i_ap, g_ap)
                    nc.vector.tensor_mul(vd[:, co : co + cw], f_ap, cd[:, co : co + cw])
                    nc.vector.tensor_add(
                        cd[:, co : co + cw], ud[:, co : co + cw], vd[:, co : co + cw]
                    )
                    tcd = decp.tile([128, 80], F32, tag="tcd")
                    nc.scalar.activation(tcd[:, co : co + cw], cd[:, co : co + cw], TANH)
                    if has_l1:
                        nc.vector.tensor_mul(
                            hd_stage[:, 0:40],
                            cap(sgd, 20, [[30, 4], [1, 10]]),
                            tcd[:, 0:40],
                        )
                    if has_dec1:
                        nc.vector.tensor_mul(
                            outbuf[:, 80 + t2 * 40 : 120 + t2 * 40],
                            cap(sgd, 140, [[30, 4], [1, 10]]),
                            tcd[:, 40:80],
                        )

            # ---- final sigmoid + writeback ----
            sig_out = pers.tile([128, nsteps * 40], F32)
            nc.scalar.activation(sig_out[:], outbuf[:, 80 : 80 + nsteps * 40], SIG)
            nc.sync.dma_start(yout[:], sig_out[:])

    nc.compile()
    return nc


# ------------------------------------------------------------------ host ---


def _perm(w, h):
    """reorder torch gate blocks [i,f,g,o] -> [i,f,o,g] along axis 0"""
    idx = np.r_[0:h, h : 2 * h, 3 * h : 4 * h, 2 * h : 3 * h]
    return w[idx]


def _prep_weights(p):
    f32 = np.float32
    out = {}
    b0 = _perm(p["te_bih0"] + p["te_bhh0"], H)
    out["w0h"] = np.ascontiguousarray(_perm(p["te_Whh0"], H).T, f32)
    out["w0xe"] = np.ascontiguousarray(
        np.vstack([_perm(p["te_Wih0"], H).T, b0[None, :]]), f32
    )
    out["w1x"] = np.ascontiguousarray(_perm(p["te_Wih1"], H).T, f32)
    out["w1h"] = np.ascontiguousarray(_perm(p["te_Whh1"], H).T, f32)
    b1 = _perm(p["te_bih1"] + p["te_bhh1"], H)
    out["b1c"] = np.ascontiguousarray(b1.reshape(4, H).T, f32)
    mlp = [
        ("ge1_W", "ge1_b"), ("ge2_W", "ge2_b"), ("ge_fc_W", "ge_fc_b"),
        ("gd_fc_W", "gd_fc_b"), ("gd1_W", "gd1_b"), ("gd2_W", "gd2_b"),
    ]
    for i, (wn, bn) in enumerate(mlp):
        out[f"mw{i}"] = np.ascontiguousarray(p[wn].T, f32)
        out[f"mb{i}"] = np.ascontiguousarray(p[bn][:, None], f32)
    bd0 = _perm(p["td_bih0"] + p["td_bhh0"], F)
    bd1 = _perm(p["td_bih1"] + p["td_bhh1"], F)
    def blkdiag(w):  # [10, 40] -> [40, 160] block-diagonal over 4 row tiles
        z = np.zeros((40, 160), f32)
        for k in range(4):
            z[k * 10 : (k + 1) * 10, k * 40 : (k + 1) * 40] = w
        return z

    wd0h = _perm(p["td_Whh0"], F).T
    wd1x = _perm(p["td_Wih1"], F).T
    wd1h = _perm(p["td_Whh1"], F).T
    out["wd0x"] = np.ascontiguousarray(_perm(p["td_Wih0"], F).T, f32)
    out["wd0h_blk"] = blkdiag(wd0h)
    out["wd1xh_blk"] = np.vstack([blkdiag(wd1x), blkdiag(wd1h)])
    out["bd0t"] = np.ascontiguousarray(
        np.tile(np.tile(bd0 / 10.0, 4)[None, :], (10, 1)), f32
    )
    out["bd1t"] = np.ascontiguousarray(
        np.tile(np.tile(bd1 / 10.0, 4)[None, :], (10, 1)), f32
    )
    out["ones10"] = np.ones((10, 128), f32)
    out["ident"] = np.eye(128, dtype=f32)
    out["zeros"] = np.zeros((H, R), f32)
    return out


def _lstm2_np(x, Wih0, Whh0, bih0, bhh0, Wih1, Whh1, bih1, bhh1):
    def layer(inp, Wih, Whh, bih, bhh):
        Rr, Tt, _ = inp.shape
        Hh = Whh.shape[1]
        xW = inp @ Wih.T + (bih + bhh)
        h = np.zeros((Rr, Hh))
        c = np.zeros((Rr, Hh))
        outs = np.empty((Rr, Tt, Hh))
        for t in range(Tt):
            g = xW[:, t] + h @ Whh.T
            i = 1 / (1 + np.exp(-g[:, :Hh]))
            f = 1 / (1 + np.exp(-g[:, Hh : 2 * Hh]))
            gg = np.tanh(g[:, 2 * Hh : 3 * Hh])
            o = 1 / (1 + np.exp(-g[:, 3 * Hh :]))
            c = f * c + i * gg
            h = o * np.tanh(c)
            outs[:, t] = h
        return outs

    return layer(layer(x, Wih0, Whh0, bih0, bhh0), Wih1, Whh1, bih1, bhh1)


def _gat_np(x, src, dst, mult, W, a_s, a_d, b):
    heads, od = a_s.shape
    h = (x @ W.T).reshape(-1, heads, od)
    es = (h * a_s[None]).sum(-1)
    ed = (h * a_d[None]).sum(-1)
    e_self = es + ed
    e_self = np.where(e_self >= 0, e_self, 0.2 * e_self)
    eb = es[src] + ed[dst]
    eb = np.where(eb >= 0, eb, 0.2 * eb)
    m = e_self.copy()
    np.maximum.at(m, dst, eb)
    ex_self = np.exp(e_self - m)
    ex_b = np.exp(eb - m[dst]) * float(mult)
    den = ex_self.copy()
    np.add.at(den, dst, ex_b)
    num = ex_self[..., None] * h
    np.add.at(num, dst, ex_b[..., None] * h[src])
    return (num / den[..., None]).reshape(-1, heads * od) + b


def _host_correction(p, out):
    """Recompute out[n=0:2, b=0] exactly (f64) with the true GAT messages."""
    f64 = np.float64
    g = lambda k: np.asarray(p[k], f64)
    x2 = g("x")[0:2, 0]  # [2, T, F]
    emb2 = g("emb")[0:2]
    hin = np.concatenate(
        [x2, np.broadcast_to(emb2[:, None, :], (2, T, EMB))], -1
    )  # [2, T, 26]
    th2 = _lstm2_np(
        hin, g("te_Wih0"), g("te_Whh0"), g("te_bih0"), g("te_bhh0"),
        g("te_Wih1"), g("te_Whh1"), g("te_bih1"), g("te_bhh1"),
    )  # [2, T, H]
    xf = th2.reshape(2 * T, H)  # rows q = n*T + t, q < 96
    src = np.asarray(p["distance_adj"])[0].astype(np.int64)
    dst = np.asarray(p["distance_adj"])[1].astype(np.int64)
    relu = lambda v: np.maximum(v, 0.0)
    mult = B * T
    hh = relu(_gat_np(xf, src, dst, mult, g("ge1_W"),
                      g("ge1_asrc"), g("ge1_adst"), g("ge1_b")))
    hh = relu(_gat_np(hh, src, dst, mult, g("ge2_W"),
                      g("ge2_asrc"), g("ge2_adst"), g("ge2_b")))
    z = relu(hh @ g("ge_fc_W").T + g("ge_fc_b"))
    hh = relu(z @ g("gd_fc_W").T + g("gd_fc_b"))
    hh = relu(_gat_np(hh, src, dst, mult, g("gd1_W"),
                      g("gd1_asrc"), g("gd1_adst"), g("gd1_b")))
    y = relu(_gat_np(hh, src, dst, mult, g("gd2_W"),
                     g("gd2_asrc"), g("gd2_adst"), g("gd2_b")))  # [96, H]
    yd = y.reshape(2, T, H)
    o2 = _lstm2_np(
        yd, g("td_Wih0"), g("td_Whh0"), g("td_bih0"), g("td_bhh0"),
        g("td_Wih1"), g("td_Whh1"), g("td_bih1"), g("td_bhh1"),
    )
    out[0:2, 0] = (1 / (1 + np.exp(-o2))).astype(np.float32)


# ---------------------------------------------------------------- kernel ---


def kernel(**inputs):
    from concourse.bass_utils import run_bass_kernel_spmd

    p = {k: np.asarray(v) for k, v in inputs.items()}
    if "nc" not in _CACHE:
        _CACHE["nc"] = _build_module(T)
    nc = _CACHE["nc"]

    w = _prep_weights(p)
    x = np.asarray(p["x"], np.float32)  # [N, B, T, F]
    xt = np.ascontiguousarray(x.transpose(3, 2, 0, 1))  # [F, T, N, B]
    embr = np.ascontiguousarray(
        np.vstack(
            [
                np.repeat(
                    np.asarray(p["emb"], np.float32).T[:, :, None], BPC, axis=2
                ).reshape(EMB, R),
                np.ones((1, R), np.float32),
            ]
        )
    )
    in_maps = []
    for i in range(NCORES):
        m = dict(w)
        m["embr"] = embr
        m["xin"] = np.ascontiguousarray(
            xt[:, :, :, i * BPC : (i + 1) * BPC].reshape(F, T, R)
        )
        in_maps.append(m)

    res = run_bass_kernel_spmd(nc, in_maps, core_ids=list(range(NCORES)))

    out = np.empty((N, B, T, F), np.float32)
    for i in range(NCORES):
        yo = res.results[i]["yout"]  # [128, T*40]
        rows = (
            yo.reshape(128, T, 4, F).transpose(2, 0, 1, 3).reshape(R, T, F)
        )  # r = k*128+p = n*8+b'
        out[:, i * BPC : (i + 1) * BPC] = rows.reshape(N, BPC, T, F)

    _host_correction(p, out)
    return out


# revision 36
# speedup vs baseline: 2.0819x; 2.0819x over previous
"""AttentionSTAE on 8 Trainium2 cores (Bass/Tile).

Structure (hardcoded shapes: N=64, B=64, T=48, F=10, EMB=16, H=128, E=256):

  Sharded data-parallel over batch B: core i handles batch slice
  b in [8i, 8i+8) -> 512 LSTM rows (r = n*8 + b'), pipelined over t:

    enc LSTM L0 (feature-major, gates [i|f|o|g] in PSUM)
    -> enc LSTM L1 (lagged one step)
    -> 6-layer dense per-node MLP (the GAT collapses to a dense layer for
       every node except global graph rows q<64; see host correction below)
    -> dec LSTM L0/L1 (row-major, lagged) -> sigmoid -> out

  The reference tiles the same [2,256] edge list (ids < 64) B*T times
  without offsets, so only global rows q<64 (batch b=0 -> core 0) receive
  real messages; every other node's GAT output is x @ W.T + b. The exact
  GAT path for the 96 affected rows (q<96 covers every decoder row that
  sees a corrected value) is recomputed on the host in float64 and the two
  affected output rows (n=0,1 at b=0) are overwritten.
"""

import numpy as np

N, B, T, F, EMB, H, E = 64, 64, 48, 10, 16, 128, 256
NCORES = 8
BPC = B // NCORES  # batch per core
R = N * BPC  # 512 LSTM rows per core
NEG = np.float32(0.2)

_CACHE = {}


# ---------------------------------------------------------------- device ---


def _build_module(nsteps=T):
    import concourse.bacc as bacc
    import concourse.tile as tile
    from concourse import mybir

    F32 = mybir.dt.float32
    F32R = mybir.dt.float32r
    SIG = mybir.ActivationFunctionType.Sigmoid
    TANH = mybir.ActivationFunctionType.Tanh
    ADD = mybir.AluOpType.add
    MAX = mybir.AluOpType.max
    import concourse.bass as bass

    nc = bacc.Bacc("TRN2", target_bir_lowering=False, debug=False)

    def din(name, shape):
        return nc.dram_tensor(name, shape, F32, kind="ExternalInput")

    xin = din("xin", [F, nsteps, R])
    embr = din("embr", [EMB + 1, R])  # emb rows + ones row (bias trick)
    w0h = din("w0h", [H, 4 * H])
    w0xe = din("w0xe", [F + EMB + 1, 4 * H])
    w1x = din("w1x", [H, 4 * H])
    w1h = din("w1h", [H, 4 * H])
    b1c = din("b1c", [H, 4])
    mlp_dims = [(H, H), (H, 64), (64, 32), (32, 64), (64, H), (H, H)]
    mws = [din(f"mw{i}", [k, m]) for i, (k, m) in enumerate(mlp_dims)]
    mbs = [din(f"mb{i}", [m, 1]) for i, (k, m) in enumerate(mlp_dims)]
    wd0x = din("wd0x", [H, 40])
    wd0h_blk = din("wd0h_blk", [40, 160])
    wd1xh_blk = din("wd1xh_blk", [80, 160])
    bd0t = din("bd0t", [10, 160])
    bd1t = din("bd1t", [10, 160])
    ones10 = din("ones10", [10, 128])
    ident = din("ident", [H, H])
    zeros = din("zeros", [H, R])
    yout = nc.dram_tensor("yout", [128, nsteps * 40], F32, kind="ExternalOutput")

    def cap(base, col_off, dims, part=None):
        """Custom AP over a tile: base partition dim + free dims (elem units)."""
        b = base[:] if not isinstance(base, bass.AP) else base
        pdim = [list(b.ap[0])]
        if part is not None:
            pdim = [[b.ap[0][0], part]]
        return bass.AP(b.tensor, b.offset + col_off, pdim + [list(d) for d in dims])

    with tile.TileContext(nc) as tc:
        with (
            tc.tile_pool(name="pers", bufs=1) as pers,
            tc.tile_pool(name="s0p", bufs=3) as s0p,
            tc.tile_pool(name="s1p", bufs=3) as s1p,
            tc.tile_pool(name="tcp", bufs=3) as tcp,
            tc.tile_pool(name="uvp", bufs=6) as uvp,
            tc.tile_pool(name="mlpa", bufs=3) as mlpa,
            tc.tile_pool(name="decp", bufs=2) as decp,
            tc.tile_pool(name="pIF", bufs=1, space="PSUM") as pIF,
            tc.tile_pool(name="pOG", bufs=1, space="PSUM") as pOG,
            tc.tile_pool(name="pMLP", bufs=2, space="PSUM") as pMLP,
            tc.tile_pool(name="pDEC", bufs=2, space="PSUM") as pDEC,
        ):
            # ---- persistent tiles ----
            h0 = pers.tile([H, R], F32R)
            h1 = pers.tile([H, R], F32R)
            c01 = pers.tile([H, 2 * R], F32)
            cd = pers.tile([128, 80], F32)
            outbuf = pers.tile([128, 80 + nsteps * 40], F32R)
            hd_stage = pers.tile([128, 80], F32R)  # [hd0(t1-1) | hd1(t2-1)]
            hdT = pers.tile([80, 128], F32R)
            xeA = pers.tile([F + EMB + 1, R], F32R)
            xeB = pers.tile([F + EMB + 1, R], F32R)
            w0h_s = pers.tile([H, 4 * H], F32R)
            w0xe_s = pers.tile([F + EMB + 1, 4 * H], F32R)
            w1x_s = pers.tile([H, 4 * H], F32R)
            w1h_s = pers.tile([H, 4 * H], F32R)
            b1_s = pers.tile([H, 4], F32)
            mw_s = [
                pers.tile([k, m], F32R, name=f"mws{i}")
                for i, (k, m) in enumerate(mlp_dims)
            ]
            mb_s = [
                pers.tile([m, 1], F32, name=f"mbs{i}")
                for i, (k, m) in enumerate(mlp_dims)
            ]
            wd0x_s = pers.tile([H, 40], F32R)
            wd0h_s = pers.tile([40, 160], F32R)
            wd1xh_s = pers.tile([80, 160], F32R)
            bd0t_s = pers.tile([10, 160], F32R)
            bd1t_s = pers.tile([10, 160], F32R)
            ones10_s = pers.tile([10, 128], F32R)
            ident_s = pers.tile([H, H], F32R)

            r32 = lambda ap: ap.bitcast(F32R)
            for dst, src in [
                (w0h_s, w0h), (w0xe_s, w0xe), (w1x_s, w1x), (w1h_s, w1h),
                (wd0x_s, wd0x), (wd0h_s, wd0h_blk), (wd1xh_s, wd1xh_blk),
                (bd0t_s, bd0t), (bd1t_s, bd1t), (ones10_s, ones10),
                (ident_s, ident),
            ]:
                nc.sync.dma_start(dst[:], r32(src[:]))
            nc.sync.dma_start(b1_s[:], b1c[:])
            for i in range(6):
                nc.sync.dma_start(mw_s[i][:], r32(mws[i][:]))
                nc.sync.dma_start(mb_s[i][:], mbs[i][:])
            nc.sync.dma_start(xeA[F : F + EMB + 1, :], r32(embr[:]))
            nc.sync.dma_start(xeB[F : F + EMB + 1, :], r32(embr[:]))

            nc.sync.dma_start(h0[:], r32(zeros[:]))
            nc.sync.dma_start(h1[:], r32(zeros[:]))
            nc.sync.dma_start(hd_stage[:], r32(zeros[:, 0:80]))
            nc.gpsimd.memset(c01[:], 0.0)
            nc.gpsimd.memset(cd[:], 0.0)


            KS = nsteps + 2  # python pipeline iters
            for k in range(KS):
                has_l0 = k < nsteps
                has_l1 = 1 <= k <= nsteps
                has_dec1 = k >= 2
                t0 = k  # enc L0 step
                t1 = k - 1  # enc L1 / MLP / dec L0 step
                t2 = k - 2  # dec L1 step

                # ---------------- encoder layer 0 (t0) ----------------
                if has_l0:
                    xe = xeA if k % 2 == 0 else xeB
                    nc.sync.dma_start(
                        xe[0:F, :], r32(xin[:, t0, :])
                    )
                    p0if = pIF.tile([H, 2 * R], F32, tag="if", name="p0if")
                    p0og = pOG.tile([H, 2 * R], F32, tag="og", name="p0og")
                    for m in range(4):
                        reg = (p0if if m < 2 else p0og)[:, (m % 2) * R : (m % 2) * R + R]
                        nc.tensor.matmul(
                            reg, w0h_s[:, m * H : (m + 1) * H], h0[:],
                            start=True, stop=False,
                        )
                        nc.tensor.matmul(
                            reg, w0xe_s[:, m * H : (m + 1) * H], xe[:],
                            start=False, stop=True,
                        )
                    s0 = s0p.tile([H, 4 * R], F32, tag="s0")
                    nc.scalar.activation(s0[:, 0 : 2 * R], p0if[:], SIG)
                    nc.scalar.activation(s0[:, 3 * R : 4 * R], p0og[:, R : 2 * R], TANH)
                    nc.scalar.activation(s0[:, 2 * R : 3 * R], p0og[:, 0:R], SIG)
                    u0 = uvp.tile([H, R], F32, tag="uv")
                    v0 = uvp.tile([H, R], F32, tag="uv")
                    nc.gpsimd.tensor_mul(v0[:], s0[:, R : 2 * R], c01[:, 0:R])
                    nc.vector.tensor_mul(u0[:], s0[:, 0:R], s0[:, 3 * R : 4 * R])
                    nc.vector.tensor_add(c01[:, 0:R], u0[:], v0[:])

                # ---------------- encoder layer 1 (t1) ----------------
                if has_l1:
                    p1if = pIF.tile([H, 2 * R], F32, tag="if", name="p1if")
                    p1og = pOG.tile([H, 2 * R], F32, tag="og", name="p1og")
                    for m in range(4):
                        reg = (p1if if m < 2 else p1og)[:, (m % 2) * R : (m % 2) * R + R]
                        nc.tensor.matmul(
                            reg, w1h_s[:, m * H : (m + 1) * H], h1[:],
                            start=True, stop=False,
                        )
                        nc.tensor.matmul(
                            reg, w1x_s[:, m * H : (m + 1) * H], h0[:],
                            start=False, stop=True,
                        )
                    s1 = s1p.tile([H, 4 * R], F32, tag="s1")
                    nc.scalar.activation(s1[:, R : 2 * R], p1if[:, R : 2 * R], SIG, bias=b1_s[:, 1:2])
                    nc.scalar.activation(s1[:, 0:R], p1if[:, 0:R], SIG, bias=b1_s[:, 0:1])
                    nc.scalar.activation(s1[:, 3 * R : 4 * R], p1og[:, R : 2 * R], TANH, bias=b1_s[:, 3:4])
                    nc.scalar.activation(s1[:, 2 * R : 3 * R], p1og[:, 0:R], SIG, bias=b1_s[:, 2:3])
                    u1 = uvp.tile([H, R], F32, tag="uv")
                    v1 = uvp.tile([H, R], F32, tag="uv")
                    nc.gpsimd.tensor_mul(v1[:], s1[:, R : 2 * R], c01[:, R : 2 * R])
                    nc.vector.tensor_mul(u1[:], s1[:, 0:R], s1[:, 3 * R : 4 * R])
                    nc.vector.tensor_add(c01[:, R : 2 * R], u1[:], v1[:])

                # tanh(c) split per layer: keeps the L0 recurrence loop short
                tc01 = tcp.tile([H, 2 * R], F32, tag="tc")
                if has_l0:
                    nc.scalar.activation(tc01[:, 0:R], c01[:, 0:R], TANH)
                    nc.gpsimd.tensor_mul(h0[:], s0[:, 2 * R : 3 * R], tc01[:, 0:R])
                if has_l1:
                    nc.scalar.activation(tc01[:, R : 2 * R], c01[:, R : 2 * R], TANH)
                    nc.gpsimd.tensor_mul(h1[:], s1[:, 2 * R : 3 * R], tc01[:, R : 2 * R])

                # ---------------- MLP (t1) ----------------
                if has_l1:
                    pm = pMLP.tile([H, R], F32, tag="mlp", name="pm")
                    prev = None
                    for i, (kk, mm) in enumerate(mlp_dims):
                        rhs = h1[:] if i == 0 else prev[0:kk, :]
                        reg = pm[0:mm, :]
                        nc.tensor.matmul(reg, mw_s[i][:], rhs, start=True, stop=True)
                        a = mlpa.tile([H, R], F32R, tag=f"a{i % 2}")
                        nc.vector.tensor_scalar(
                            a[0:mm, :], reg, mb_s[i][:, 0:1], 0.0, ADD, MAX
                        )
                        prev = a
                    y_t = prev  # [128, R] f32r tile

                # ---------------- decoder (dec L0 @ t1, dec L1 @ t2) ----------
                if has_l1 or has_dec1:
                    pd = pDEC.tile([H, R], F32, tag="dec", name="pd")
                    # transpose [hd0(t1-1) | hd1(t2-1)] -> hdT
                    if k >= 3:
                        nc.vector.tensor_copy(
                            hd_stage[:, 40:80],
                            outbuf[:, 80 + (k - 3) * 40 : 120 + (k - 3) * 40],
                        )
                    tps = cap(pd, 320, [[1, 128]], part=80).bitcast(F32R)
                    nc.tensor.transpose(tps, hd_stage[:], ident_s[:])
                    nc.vector.tensor_copy(hdT[:], tps)

                if has_l1:  # dec L0 at t1
                    nc.tensor.matmul(
                        pd[:, 0:160], ones10_s[:], bd0t_s[:],
                        start=True, stop=False, skip_group_check=True,
                    )
                    for kk in range(4):
                        nc.tensor.matmul(
                            pd[:, kk * 40 : kk * 40 + 40],
                            y_t[:, kk * 128 : (kk + 1) * 128],
                            wd0x_s[:], start=False, stop=False,
                            skip_group_check=True,
                        )
                    nc.tensor.matmul(
                        pd[:, 0:160], hdT[0:40, :], wd0h_s[:],
                        start=False, stop=True, skip_group_check=True,
                    )
                if has_dec1:  # dec L1 at t2 (x from hd0, h from hd1: one mm)
                    nc.tensor.matmul(
                        pd[:, 160:320], hdT[:], wd1xh_s[:],
                        start=True, stop=False, skip_group_check=True,
                    )
                    nc.tensor.matmul(
                        pd[:, 160:320], ones10_s[:], bd1t_s[:],
                        start=False, stop=True, skip_group_check=True,
                    )

                if has_l1 or has_dec1:
                    lt_off = 0 if has_l1 else 4  # active lt range
                    lt_cnt = (4 if has_l1 else 0) + (4 if has_dec1 else 0)
                    sgd = decp.tile([128, 240], F32, tag="sgd")
                    tgd = decp.tile([128, 80], F32, tag="tgd")
                    po, so = lt_off * 40, lt_off * 30
                    nc.scalar.activation(
                        cap(sgd, so, [[30, lt_cnt], [1, 30]]),
                        cap(pd, po, [[40, lt_cnt], [1, 30]]), SIG,
                    )
                    nc.scalar.activation(
                        cap(tgd, lt_off * 10, [[10, lt_cnt], [1, 10]]),
                        cap(pd, po + 30, [[40, lt_cnt], [1, 10]]), TANH,
                    )
                    ud = decp.tile([128, 80], F32, tag="ud")
                    vd = decp.tile([128, 80], F32, tag="vd")
                    co = lt_off * 10
                    cw = lt_cnt * 10
                    i_ap = cap(sgd, so, [[30, lt_cnt], [1, 10]])
                    f_ap = cap(sgd, so + 10, [[30, lt_cnt], [1, 10]])
                    o_ap = cap(sgd, so + 20, [[30, lt_cnt], [1, 10]])
                    g_ap = cap(tgd, co, [[10, lt_cnt], [1, 10]])
                    nc.vector.tensor_mul(ud[:, co : co + cw], i_ap, g_ap)
                    nc.vector.tensor_mul(vd[:, co : co + cw], f_ap, cd[:, co : co + cw])
                    nc.vector.tensor_add(
                        cd[:, co : co + cw], ud[:, co : co + cw], vd[:, co : co + cw]
                    )
                    tcd = decp.tile([128, 80], F32, tag="tcd")
                    nc.scalar.activation(tcd[:, co : co + cw], cd[:, co : co + cw], TANH)
                    if has_l1:
                        nc.vector.tensor_mul(
                            hd_stage[:, 0:40],
                            cap(sgd, 20, [[30, 4], [1, 10]]),
                            tcd[:, 0:40],
                        )
                    if has_dec1:
                        nc.vector.tensor_mul(
                            outbuf[:, 80 + t2 * 40 : 120 + t2 * 40],
                            cap(sgd, 140, [[30, 4], [1, 10]]),
                            tcd[:, 40:80],
                        )

            # ---- final sigmoid + writeback ----
            sig_out = pers.tile([128, nsteps * 40], F32)
            nc.scalar.activation(sig_out[:], outbuf[:, 80 : 80 + nsteps * 40], SIG)
            nc.sync.dma_start(yout[:], sig_out[:])

    nc.compile()
    return nc


# ------------------------------------------------------------------ host ---


def _perm(w, h):
    """reorder torch gate blocks [i,f,g,o] -> [i,f,o,g] along axis 0"""
    idx = np.r_[0:h, h : 2 * h, 3 * h : 4 * h, 2 * h : 3 * h]
    return w[idx]


def _prep_weights(p):
    f32 = np.float32
    out = {}
    b0 = _perm(p["te_bih0"] + p["te_bhh0"], H)
    out["w0h"] = np.ascontiguousarray(_perm(p["te_Whh0"], H).T, f32)
    out["w0xe"] = np.ascontiguousarray(
        np.vstack([_perm(p["te_Wih0"], H).T, b0[None, :]]), f32
    )
    out["w1x"] = np.ascontiguousarray(_perm(p["te_Wih1"], H).T, f32)
    out["w1h"] = np.ascontiguousarray(_perm(p["te_Whh1"], H).T, f32)
    b1 = _perm(p["te_bih1"] + p["te_bhh1"], H)
    out["b1c"] = np.ascontiguousarray(b1.reshape(4, H).T, f32)
    mlp = [
        ("ge1_W", "ge1_b"), ("ge2_W", "ge2_b"), ("ge_fc_W", "ge_fc_b"),
        ("gd_fc_W", "gd_fc_b"), ("gd1_W", "gd1_b"), ("gd2_W", "gd2_b"),
    ]
    for i, (wn, bn) in enumerate(mlp):
        out[f"mw{i}"] = np.ascontiguousarray(p[wn].T, f32)
        out[f"mb{i}"] = np.ascontiguousarray(p[bn][:, None], f32)
    bd0 = _perm(p["td_bih0"] + p["td_bhh0"], F)
    bd1 = _perm(p["td_bih1"] + p["td_bhh1"], F)
    def blkdiag(w):  # [10, 40] -> [40, 160] block-diagonal over 4 row tiles
        z = np.zeros((40, 160), f32)
        for k in range(4):
            z[k * 10 : (k + 1) * 10, k * 40 : (k + 1) * 40] = w
        return z

    wd0h = _perm(p["td_Whh0"], F).T
    wd1x = _perm(p["td_Wih1"], F).T
    wd1h = _perm(p["td_Whh1"], F).T
    out["wd0x"] = np.ascontiguousarray(_perm(p["td_Wih0"], F).T, f32)
    out["wd0h_blk"] = blkdiag(wd0h)
    out["wd1xh_blk"] = np.vstack([blkdiag(wd1x), blkdiag(wd1h)])
    out["bd0t"] = np.ascontiguousarray(
        np.tile(np.tile(bd0 / 10.0, 4)[None, :], (10, 1)), f32
    )
    out["bd1t"] = np.ascontiguousarray(
        np.tile(np.tile(bd1 / 10.0, 4)[None, :], (10, 1)), f32
    )
    out["ones10"] = np.ones((10, 128), f32)
    out["ident"] = np.eye(128, dtype=f32)
    out["zeros"] = np.zeros((H, R), f32)
    return out


def _lstm2_np(x, Wih0, Whh0, bih0, bhh0, Wih1, Whh1, bih1, bhh1):
    def layer(inp, Wih, Whh, bih, bhh):
        Rr, Tt, _ = inp.shape
        Hh = Whh.shape[1]
        xW = inp @ Wih.T + (bih + bhh)
        h = np.zeros((Rr, Hh))
        c = np.zeros((Rr, Hh))
        outs = np.empty((Rr, Tt, Hh))
        for t in range(Tt):
            g = xW[:, t] + h @ Whh.T
            i = 1 / (1 + np.exp(-g[:, :Hh]))
            f = 1 / (1 + np.exp(-g[:, Hh : 2 * Hh]))
            gg = np.tanh(g[:, 2 * Hh : 3 * Hh])
            o = 1 / (1 + np.exp(-g[:, 3 * Hh :]))
            c = f * c + i * gg
            h = o * np.tanh(c)
            outs[:, t] = h
        return outs

    return layer(layer(x, Wih0, Whh0, bih0, bhh0), Wih1, Whh1, bih1, bhh1)


def _gat_np(x, src, dst, mult, W, a_s, a_d, b):
    heads, od = a_s.shape
    h = (x @ W.T).reshape(-1, heads, od)
    es = (h * a_s[None]).sum(-1)
    ed = (h * a_d[None]).sum(-1)
    e_self = es + ed
    e_self = np.where(e_self >= 0, e_self, 0.2 * e_self)
    eb = es[src] + ed[dst]
    eb = np.where(eb >= 0, eb, 0.2 * eb)
    m = e_self.copy()
    np.maximum.at(m, dst, eb)
    ex_self = np.exp(e_self - m)
    ex_b = np.exp(eb - m[dst]) * float(mult)
    den = ex_self.copy()
    np.add.at(den, dst, ex_b)
    num = ex_self[..., None] * h
    np.add.at(num, dst, ex_b[..., None] * h[src])
    return (num / den[..., None]).reshape(-1, heads * od) + b


def _host_correction(p, out):
    """Recompute out[n=0:2, b=0] exactly (f64) with the true GAT messages."""
    f64 = np.float64
    g = lambda k: np.asarray(p[k], f64)
    x2 = g("x")[0:2, 0]  # [2, T, F]
    emb2 = g("emb")[0:2]
    hin = np.concatenate(
        [x2, np.broadcast_to(emb2[:, None, :], (2, T, EMB))], -1
    )  # [2, T, 26]
    th2 = _lstm2_np(
        hin, g("te_Wih0"), g("te_Whh0"), g("te_bih0"), g("te_bhh0"),
        g("te_Wih1"), g("te_Whh1"), g("te_bih1"), g("te_bhh1"),
    )  # [2, T, H]
    xf = th2.reshape(2 * T, H)  # rows q = n*T + t, q < 96
    src = np.asarray(p["distance_adj"])[0].astype(np.int64)
    dst = np.asarray(p["distance_adj"])[1].astype(np.int64)
    relu = lambda v: np.maximum(v, 0.0)
    mult = B * T
    hh = relu(_gat_np(xf, src, dst, mult, g("ge1_W"),
                      g("ge1_asrc"), g("ge1_adst"), g("ge1_b")))
    hh = relu(_gat_np(hh, src, dst, mult, g("ge2_W"),
                      g("ge2_asrc"), g("ge2_adst"), g("ge2_b")))
    z = relu(hh @ g("ge_fc_W").T + g("ge_fc_b"))
    hh = relu(z @ g("gd_fc_W").T + g("gd_fc_b"))
    hh = relu(_gat_np(hh, src, dst, mult, g("gd1_W"),
                      g("gd1_asrc"), g("gd1_adst"), g("gd1_b")))
    y = relu(_gat_np(hh, src, dst, mult, g("gd2_W"),
                     g("gd2_asrc"), g("gd2_adst"), g("gd2_b")))  # [96, H]
    yd = y.reshape(2, T, H)
    o2 = _lstm2_np(
        yd, g("td_Wih0"), g("td_Whh0"), g("td_bih0"), g("td_bhh0"),
        g("td_Wih1"), g("td_Whh1"), g("td_bih1"), g("td_bhh1"),
    )
    out[0:2, 0] = (1 / (1 + np.exp(-o2))).astype(np.float32)


# ---------------------------------------------------------------- kernel ---


def kernel(**inputs):
    from concourse.bass_utils import run_bass_kernel_spmd

    p = {k: np.asarray(v) for k, v in inputs.items()}
    if "nc" not in _CACHE:
        _CACHE["nc"] = _build_module(T)
    nc = _CACHE["nc"]

    w = _prep_weights(p)
    x = np.asarray(p["x"], np.float32)  # [N, B, T, F]
    xt = np.ascontiguousarray(x.transpose(3, 2, 0, 1))  # [F, T, N, B]
    embr = np.ascontiguousarray(
        np.vstack(
            [
                np.repeat(
                    np.asarray(p["emb"], np.float32).T[:, :, None], BPC, axis=2
                ).reshape(EMB, R),
                np.ones((1, R), np.float32),
            ]
        )
    )
    in_maps = []
    for i in range(NCORES):
        m = dict(w)
        m["embr"] = embr
        m["xin"] = np.ascontiguousarray(
            xt[:, :, :, i * BPC : (i + 1) * BPC].reshape(F, T, R)
        )
        in_maps.append(m)

    res = run_bass_kernel_spmd(nc, in_maps, core_ids=list(range(NCORES)))

    out = np.empty((N, B, T, F), np.float32)
    for i in range(NCORES):
        yo = res.results[i]["yout"]  # [128, T*40]
        rows = (
            yo.reshape(128, T, 4, F).transpose(2, 0, 1, 3).reshape(R, T, F)
        )  # r = k*128+p = n*8+b'
        out[:, i * BPC : (i + 1) * BPC] = rows.reshape(N, BPC, T, F)

    _host_correction(p, out)
    return out


# revision 44
# speedup vs baseline: 2.1373x; 1.0266x over previous
"""AttentionSTAE on 8 Trainium2 cores (Bass/Tile).

Structure (hardcoded shapes: N=64, B=64, T=48, F=10, EMB=16, H=128, E=256):

  Sharded data-parallel over batch B: core i handles batch slice
  b in [8i, 8i+8) -> 512 LSTM rows (r = n*8 + b'), pipelined over t:

    enc LSTM L0 (feature-major, gates [i|f|o|g] in PSUM)
    -> enc LSTM L1 (lagged one step)
    -> 6-layer dense per-node MLP (the GAT collapses to a dense layer for
       every node except global graph rows q<64; see host correction below)
    -> dec LSTM L0/L1 (row-major, lagged) -> sigmoid -> out

  The reference tiles the same [2,256] edge list (ids < 64) B*T times
  without offsets, so only global rows q<64 (batch b=0 -> core 0) receive
  real messages; every other node's GAT output is x @ W.T + b. The exact
  GAT path for the 96 affected rows (q<96 covers every decoder row that
  sees a corrected value) is recomputed on the host in float64 and the two
  affected output rows (n=0,1 at b=0) are overwritten.
"""

import numpy as np

N, B, T, F, EMB, H, E = 64, 64, 48, 10, 16, 128, 256
NCORES = 8
BPC = B // NCORES  # batch per core
R = N * BPC  # 512 LSTM rows per core
NEG = np.float32(0.2)

_CACHE = {}


# ---------------------------------------------------------------- device ---


def _build_module(nsteps=T):
    import concourse.bacc as bacc
    import concourse.tile as tile
    from concourse import mybir

    F32 = mybir.dt.float32
    F32R = mybir.dt.float32r
    SIG = mybir.ActivationFunctionType.Sigmoid
    TANH = mybir.ActivationFunctionType.Tanh
    ADD = mybir.AluOpType.add
    MAX = mybir.AluOpType.max
    import concourse.bass as bass

    nc = bacc.Bacc("TRN2", target_bir_lowering=False, debug=False)

    def din(name, shape):
        return nc.dram_tensor(name, shape, F32, kind="ExternalInput")

    xin = din("xin", [F, nsteps, R])
    embr = din("embr", [EMB + 1, R])  # emb rows + ones row (bias trick)
    w0h = din("w0h", [H, 4 * H])
    w0xe = din("w0xe", [F + EMB + 1, 4 * H])
    w1x = din("w1x", [H, 4 * H])
    w1h = din("w1h", [H, 4 * H])
    b1c = din("b1c", [H, 4])
    mlp_dims = [(H, H), (H, 64), (64, 32), (32, 64), (64, H), (H, H)]
    mws = [din(f"mw{i}", [k, m]) for i, (k, m) in enumerate(mlp_dims)]
    mbs = [din(f"mb{i}", [m, 1]) for i, (k, m) in enumerate(mlp_dims)]
    wd0x = din("wd0x", [H, 40])
    wd0h_blk = din("wd0h_blk", [40, 160])
    wd1xh_blk = din("wd1xh_blk", [80, 160])
    bd0t = din("bd0t", [10, 160])
    bd1t = din("bd1t", [10, 160])
    ones10 = din("ones10", [10, 128])
    ident = din("ident", [H, H])
    zeros = din("zeros", [H, R])
    yout = nc.dram_tensor("yout", [128, nsteps * 40], F32, kind="ExternalOutput")

    def cap(base, col_off, dims, part=None):
        """Custom AP over a tile: base partition dim + free dims (elem units)."""
        b = base[:] if not isinstance(base, bass.AP) else base
        pdim = [list(b.ap[0])]
        if part is not None:
            pdim = [[b.ap[0][0], part]]
        return bass.AP(b.tensor, b.offset + col_off, pdim + [list(d) for d in dims])

    with tile.TileContext(nc) as tc:
        with (
            tc.tile_pool(name="pers", bufs=1) as pers,
            tc.tile_pool(name="s0p", bufs=3) as s0p,
            tc.tile_pool(name="s1p", bufs=3) as s1p,
            tc.tile_pool(name="tcp", bufs=3) as tcp,
            tc.tile_pool(name="uvp", bufs=6) as uvp,
            tc.tile_pool(name="mlpa", bufs=3) as mlpa,
            tc.tile_pool(name="decp", bufs=2) as decp,
            tc.tile_pool(name="pIF", bufs=2, space="PSUM") as pIF,
            tc.tile_pool(name="pOG", bufs=1, space="PSUM") as pOG,
            tc.tile_pool(name="pMLP", bufs=1, space="PSUM") as pMLP,
            tc.tile_pool(name="pDEC", bufs=1, space="PSUM") as pDEC,
        ):
            # ---- persistent tiles ----
            h0 = pers.tile([H, R], F32R)
            h1 = pers.tile([H, R], F32R)
            c01 = pers.tile([H, 2 * R], F32)
            cd = pers.tile([128, 80], F32)
            outbuf = pers.tile([128, 80 + nsteps * 40], F32R)
            hd_stage = pers.tile([128, 80], F32R)  # [hd0(t1-1) | hd1(t2-1)]
            hdT = pers.tile([80, 128], F32R)
            xeA = pers.tile([F + EMB + 1, R], F32R)
            xeB = pers.tile([F + EMB + 1, R], F32R)
            w0h_s = pers.tile([H, 4 * H], F32R)
            w0xe_s = pers.tile([F + EMB + 1, 4 * H], F32R)
            w1x_s = pers.tile([H, 4 * H], F32R)
            w1h_s = pers.tile([H, 4 * H], F32R)
            b1_s = pers.tile([H, 4], F32)
            mw_s = [
                pers.tile([k, m], F32R, name=f"mws{i}")
                for i, (k, m) in enumerate(mlp_dims)
            ]
            mb_s = [
                pers.tile([m, 1], F32, name=f"mbs{i}")
                for i, (k, m) in enumerate(mlp_dims)
            ]
            wd0x_s = pers.tile([H, 40], F32R)
            wd0h_s = pers.tile([40, 160], F32R)
            wd1xh_s = pers.tile([80, 160], F32R)
            bd0t_s = pers.tile([10, 160], F32R)
            bd1t_s = pers.tile([10, 160], F32R)
            ones10_s = pers.tile([10, 128], F32R)
            ident_s = pers.tile([H, H], F32R)

            r32 = lambda ap: ap.bitcast(F32R)
            for dst, src in [
                (w0h_s, w0h), (w0xe_s, w0xe), (w1x_s, w1x), (w1h_s, w1h),
                (wd0x_s, wd0x), (wd0h_s, wd0h_blk), (wd1xh_s, wd1xh_blk),
                (bd0t_s, bd0t), (bd1t_s, bd1t), (ones10_s, ones10),
                (ident_s, ident),
            ]:
                nc.sync.dma_start(dst[:], r32(src[:]))
            nc.sync.dma_start(b1_s[:], b1c[:])
            for i in range(6):
                nc.sync.dma_start(mw_s[i][:], r32(mws[i][:]))
                nc.sync.dma_start(mb_s[i][:], mbs[i][:])
            nc.sync.dma_start(xeA[F : F + EMB + 1, :], r32(embr[:]))
            nc.sync.dma_start(xeB[F : F + EMB + 1, :], r32(embr[:]))

            nc.sync.dma_start(h0[:], r32(zeros[:]))
            nc.sync.dma_start(h1[:], r32(zeros[:]))
            nc.sync.dma_start(hd_stage[:], r32(zeros[:, 0:80]))
            nc.gpsimd.memset(c01[:], 0.0)
            nc.gpsimd.memset(cd[:], 0.0)


            KS = nsteps + 2  # python pipeline iters
            for k in range(KS):
                has_l0 = k < nsteps
                has_l1 = 1 <= k <= nsteps
                has_dec1 = k >= 2
                t0 = k  # enc L0 step
                t1 = k - 1  # enc L1 / MLP / dec L0 step
                t2 = k - 2  # dec L1 step

                # ---------------- encoder layer 0 (t0) ----------------
                if has_l0:
                    xe = xeA if k % 2 == 0 else xeB
                    nc.sync.dma_start(
                        xe[0:F, :], r32(xin[:, t0, :])
                    )
                    p0if = pIF.tile([H, 2 * R], F32, tag="if", name="p0if")
                    p0og = pOG.tile([H, 2 * R], F32, tag="og", name="p0og")
                    p0o = p0og[:, 0:R]
                    p0g = p0og[:, R : 2 * R]
                    for m in range(4):
                        reg = (
                            p0if[:, (m % 2) * R : (m % 2) * R + R]
                            if m < 2
                            else (p0o if m == 2 else p0g)
                        )
                        nc.tensor.matmul(
                            reg, w0h_s[:, m * H : (m + 1) * H], h0[:],
                            start=True, stop=False,
                        )
                        nc.tensor.matmul(
                            reg, w0xe_s[:, m * H : (m + 1) * H], xe[:],
                            start=False, stop=True,
                        )
                    s0 = s0p.tile([H, 4 * R], F32, tag="s0")
                    nc.scalar.activation(s0[:, 0 : 2 * R], p0if[:], SIG)
                    nc.scalar.activation(s0[:, 3 * R : 4 * R], p0g, TANH)
                    nc.scalar.activation(s0[:, 2 * R : 3 * R], p0o, SIG)
                    u0 = uvp.tile([H, R], F32, tag="uv")
                    v0 = uvp.tile([H, R], F32, tag="uv")
                    nc.gpsimd.tensor_mul(v0[:], s0[:, R : 2 * R], c01[:, 0:R])
                    nc.vector.tensor_mul(u0[:], s0[:, 0:R], s0[:, 3 * R : 4 * R])
                    nc.vector.tensor_add(c01[:, 0:R], u0[:], v0[:])

                # ---------------- encoder layer 1 (t1) ----------------
                if has_l1:
                    p1if = pIF.tile([H, 2 * R], F32, tag="if", name="p1if")
                    p1og = pOG.tile([H, 2 * R], F32, tag="og", name="p1og")
                    p1o = p1og[:, 0:R]
                    p1g = p1og[:, R : 2 * R]
                    for m in range(4):
                        reg = (
                            p1if[:, (m % 2) * R : (m % 2) * R + R]
                            if m < 2
                            else (p1o if m == 2 else p1g)
                        )
                        nc.tensor.matmul(
                            reg, w1h_s[:, m * H : (m + 1) * H], h1[:],
                            start=True, stop=False,
                        )
                        nc.tensor.matmul(
                            reg, w1x_s[:, m * H : (m + 1) * H], h0[:],
                            start=False, stop=True,
                        )
                    s1 = s1p.tile([H, 4 * R], F32, tag="s1")
                    nc.scalar.activation(s1[:, R : 2 * R], p1if[:, R : 2 * R], SIG, bias=b1_s[:, 1:2])
                    nc.scalar.activation(s1[:, 0:R], p1if[:, 0:R], SIG, bias=b1_s[:, 0:1])
                    nc.scalar.activation(s1[:, 3 * R : 4 * R], p1g, TANH, bias=b1_s[:, 3:4])
                    nc.scalar.activation(s1[:, 2 * R : 3 * R], p1o, SIG, bias=b1_s[:, 2:3])
                    u1 = uvp.tile([H, R], F32, tag="uv")
                    v1 = uvp.tile([H, R], F32, tag="uv")
                    nc.gpsimd.tensor_mul(v1[:], s1[:, R : 2 * R], c01[:, R : 2 * R])
                    nc.vector.tensor_mul(u1[:], s1[:, 0:R], s1[:, 3 * R : 4 * R])
                    nc.vector.tensor_add(c01[:, R : 2 * R], u1[:], v1[:])

                # tanh(c) split per layer: keeps the L0 recurrence loop short
                tc01 = tcp.tile([H, 2 * R], F32, tag="tc")
                if has_l0:
                    nc.scalar.activation(tc01[:, 0:R], c01[:, 0:R], TANH)
                    nc.gpsimd.tensor_mul(h0[:], s0[:, 2 * R : 3 * R], tc01[:, 0:R])
                if has_l1:
                    nc.scalar.activation(tc01[:, R : 2 * R], c01[:, R : 2 * R], TANH)
                    nc.gpsimd.tensor_mul(h1[:], s1[:, 2 * R : 3 * R], tc01[:, R : 2 * R])

                # ---------------- MLP (t1) ----------------
                if has_l1:
                    pm = pMLP.tile([H, R], F32, tag="mlp", name="pm")
                    prev = None
                    for i, (kk, mm) in enumerate(mlp_dims):
                        rhs = h1[:] if i == 0 else prev[0:kk, :]
                        reg = pm[0:mm, :]
                        nc.tensor.matmul(reg, mw_s[i][:], rhs, start=True, stop=True)
                        a = mlpa.tile([H, R], F32R, tag=f"a{i % 2}")
                        nc.vector.tensor_scalar(
                            a[0:mm, :], reg, mb_s[i][:, 0:1], 0.0, ADD, MAX
                        )
                        prev = a
                    y_t = prev  # [128, R] f32r tile

                # ---------------- decoder (dec L0 @ t1, dec L1 @ t2) ----------
                if has_l1 or has_dec1:
                    pd = pDEC.tile([H, R], F32, tag="dec", name="pd")
                    # transpose [hd0(t1-1) | hd1(t2-1)] -> hdT
                    if k >= 3:
                        nc.vector.tensor_copy(
                            hd_stage[:, 40:80],
                            outbuf[:, 80 + (k - 3) * 40 : 120 + (k - 3) * 40],
                        )
                    tps = cap(pd, 320, [[1, 128]], part=80).bitcast(F32R)
                    nc.tensor.transpose(tps, hd_stage[:], ident_s[:])
                    nc.vector.tensor_copy(hdT[:], tps)

                if has_l1:  # dec L0 at t1
                    nc.tensor.matmul(
                        pd[:, 0:160], ones10_s[:], bd0t_s[:],
                        start=True, stop=False, skip_group_check=True,
                    )
                    for kk in range(4):
                        nc.tensor.matmul(
                            pd[:, kk * 40 : kk * 40 + 40],
                            y_t[:, kk * 128 : (kk + 1) * 128],
                            wd0x_s[:], start=False, stop=False,
                            skip_group_check=True,
                        )
                    nc.tensor.matmul(
                        pd[:, 0:160], hdT[0:40, :], wd0h_s[:],
                        start=False, stop=True, skip_group_check=True,
                    )
                if has_dec1:  # dec L1 at t2 (x from hd0, h from hd1: one mm)
                    nc.tensor.matmul(
                        pd[:, 160:320], hdT[:], wd1xh_s[:],
                        start=True, stop=False, skip_group_check=True,
                    )
                    nc.tensor.matmul(
                        pd[:, 160:320], ones10_s[:], bd1t_s[:],
                        start=False, stop=True, skip_group_check=True,
                    )

                if has_l1 or has_dec1:
                    lt_off = 0 if has_l1 else 4  # active lt range
                    lt_cnt = (4 if has_l1 else 0) + (4 if has_dec1 else 0)
                    sgd = decp.tile([128, 240], F32, tag="sgd")
                    tgd = decp.tile([128, 80], F32, tag="tgd")
                    po, so = lt_off * 40, lt_off * 30
                    nc.scalar.activation(
                        cap(sgd, so, [[30, lt_cnt], [1, 30]]),
                        cap(pd, po, [[40, lt_cnt], [1, 30]]), SIG,
                    )
                    nc.scalar.activation(
                        cap(tgd, lt_off * 10, [[10, lt_cnt], [1, 10]]),
                        cap(pd, po + 30, [[40, lt_cnt], [1, 10]]), TANH,
                    )
                    ud = decp.tile([128, 80], F32, tag="ud")
                    vd = decp.tile([128, 80], F32, tag="vd")
                    co = lt_off * 10
                    cw = lt_cnt * 10
                    i_ap = cap(sgd, so, [[30, lt_cnt], [1, 10]])
                    f_ap = cap(sgd, so + 10, [[30, lt_cnt], [1, 10]])
                    o_ap = cap(sgd, so + 20, [[30, lt_cnt], [1, 10]])
                    g_ap = cap(tgd, co, [[10, lt_cnt], [1, 10]])
                    nc.vector.tensor_mul(ud[:, co : co + cw], i_ap, g_ap)
                    nc.vector.tensor_mul(vd[:, co : co + cw], f_ap, cd[:, co : co + cw])
                    nc.vector.tensor_add(
                        cd[:, co : co + cw], ud[:, co : co + cw], vd[:, co : co + cw]
                    )
                    tcd = decp.tile([128, 80], F32, tag="tcd")
                    nc.scalar.activation(tcd[:, co : co + cw], cd[:, co : co + cw], TANH)
                    if has_l1:
                        nc.vector.tensor_mul(
                            hd_stage[:, 0:40],
                            cap(sgd, 20, [[30, 4], [1, 10]]),
                            tcd[:, 0:40],
                        )
                    if has_dec1:
                        nc.vector.tensor_mul(
                            outbuf[:, 80 + t2 * 40 : 120 + t2 * 40],
                            cap(sgd, 140, [[30, 4], [1, 10]]),
                            tcd[:, 40:80],
                        )

            # ---- final sigmoid + writeback ----
            sig_out = pers.tile([128, nsteps * 40], F32)
            nc.scalar.activation(sig_out[:], outbuf[:, 80 : 80 + nsteps * 40], SIG)
            nc.sync.dma_start(yout[:], sig_out[:])

    nc.compile()
    return nc


# ------------------------------------------------------------------ host ---


def _perm(w, h):
    """reorder torch gate blocks [i,f,g,o] -> [i,f,o,g] along axis 0"""
    idx = np.r_[0:h, h : 2 * h, 3 * h : 4 * h, 2 * h : 3 * h]
    return w[idx]


def _prep_weights(p):
    f32 = np.float32
    out = {}
    b0 = _perm(p["te_bih0"] + p["te_bhh0"], H)
    out["w0h"] = np.ascontiguousarray(_perm(p["te_Whh0"], H).T, f32)
    out["w0xe"] = np.ascontiguousarray(
        np.vstack([_perm(p["te_Wih0"], H).T, b0[None, :]]), f32
    )
    out["w1x"] = np.ascontiguousarray(_perm(p["te_Wih1"], H).T, f32)
    out["w1h"] = np.ascontiguousarray(_perm(p["te_Whh1"], H).T, f32)
    b1 = _perm(p["te_bih1"] + p["te_bhh1"], H)
    out["b1c"] = np.ascontiguousarray(b1.reshape(4, H).T, f32)
    mlp = [
        ("ge1_W", "ge1_b"), ("ge2_W", "ge2_b"), ("ge_fc_W", "ge_fc_b"),
        ("gd_fc_W", "gd_fc_b"), ("gd1_W", "gd1_b"), ("gd2_W", "gd2_b"),
    ]
    for i, (wn, bn) in enumerate(mlp):
        out[f"mw{i}"] = np.ascontiguousarray(p[wn].T, f32)
        out[f"mb{i}"] = np.ascontiguousarray(p[bn][:, None], f32)
    bd0 = _perm(p["td_bih0"] + p["td_bhh0"], F)
    bd1 = _perm(p["td_bih1"] + p["td_bhh1"], F)
    def blkdiag(w):  # [10, 40] -> [40, 160] block-diagonal over 4 row tiles
        z = np.zeros((40, 160), f32)
        for k in range(4):
            z[k * 10 : (k + 1) * 10, k * 40 : (k + 1) * 40] = w
        return z

    wd0h = _perm(p["td_Whh0"], F).T
    wd1x = _perm(p["td_Wih1"], F).T
    wd1h = _perm(p["td_Whh1"], F).T
    out["wd0x"] = np.ascontiguousarray(_perm(p["td_Wih0"], F).T, f32)
    out["wd0h_blk"] = blkdiag(wd0h)
    out["wd1xh_blk"] = np.vstack([blkdiag(wd1x), blkdiag(wd1h)])
    out["bd0t"] = np.ascontiguousarray(
        np.tile(np.tile(bd0 / 10.0, 4)[None, :], (10, 1)), f32
    )
    out["bd1t"] = np.ascontiguousarray(
        np.tile(np.tile(bd1 / 10.0, 4)[None, :], (10, 1)), f32
    )
    out["ones10"] = np.ones((10, 128), f32)
    out["ident"] = np.eye(128, dtype=f32)
    out["zeros"] = np.zeros((H, R), f32)
    return out


def _lstm2_np(x, Wih0, Whh0, bih0, bhh0, Wih1, Whh1, bih1, bhh1):
    def layer(inp, Wih, Whh, bih, bhh):
        Rr, Tt, _ = inp.shape
        Hh = Whh.shape[1]
        xW = inp @ Wih.T + (bih + bhh)
        h = np.zeros((Rr, Hh))
        c = np.zeros((Rr, Hh))
        outs = np.empty((Rr, Tt, Hh))
        for t in range(Tt):
            g = xW[:, t] + h @ Whh.T
            i = 1 / (1 + np.exp(-g[:, :Hh]))
            f = 1 / (1 + np.exp(-g[:, Hh : 2 * Hh]))
            gg = np.tanh(g[:, 2 * Hh : 3 * Hh])
            o = 1 / (1 + np.exp(-g[:, 3 * Hh :]))
            c = f * c + i * gg
            h = o * np.tanh(c)
            outs[:, t] = h
        return outs

    return layer(layer(x, Wih0, Whh0, bih0, bhh0), Wih1, Whh1, bih1, bhh1)


def _gat_np(x, src, dst, mult, W, a_s, a_d, b):
    heads, od = a_s.shape
    h = (x @ W.T).reshape(-1, heads, od)
    es = (h * a_s[None]).sum(-1)
    ed = (h * a_d[None]).sum(-1)
    e_self = es + ed
    e_self = np.where(e_self >= 0, e_self, 0.2 * e_self)
    eb = es[src] + ed[dst]
    eb = np.where(eb >= 0, eb, 0.2 * eb)
    m = e_self.copy()
    np.maximum.at(m, dst, eb)
    ex_self = np.exp(e_self - m)
    ex_b = np.exp(eb - m[dst]) * float(mult)
    den = ex_self.copy()
    np.add.at(den, dst, ex_b)
    num = ex_self[..., None] * h
    np.add.at(num, dst, ex_b[..., None] * h[src])
    return (num / den[..., None]).reshape(-1, heads * od) + b


def _host_correction(p, out):
    """Recompute out[n=0:2, b=0] exactly (f64) with the true GAT messages."""
    f64 = np.float64
    g = lambda k: np.asarray(p[k], f64)
    x2 = g("x")[0:2, 0]  # [2, T, F]
    emb2 = g("emb")[0:2]
    hin = np.concatenate(
        [x2, np.broadcast_to(emb2[:, None, :], (2, T, EMB))], -1
    )  # [2, T, 26]
    th2 = _lstm2_np(
        hin, g("te_Wih0"), g("te_Whh0"), g("te_bih0"), g("te_bhh0"),
        g("te_Wih1"), g("te_Whh1"), g("te_bih1"), g("te_bhh1"),
    )  # [2, T, H]
    xf = th2.reshape(2 * T, H)  # rows q = n*T + t, q < 96
    src = np.asarray(p["distance_adj"])[0].astype(np.int64)
    dst = np.asarray(p["distance_adj"])[1].astype(np.int64)
    relu = lambda v: np.maximum(v, 0.0)
    mult = B * T
    hh = relu(_gat_np(xf, src, dst, mult, g("ge1_W"),
                      g("ge1_asrc"), g("ge1_adst"), g("ge1_b")))
    hh = relu(_gat_np(hh, src, dst, mult, g("ge2_W"),
                      g("ge2_asrc"), g("ge2_adst"), g("ge2_b")))
    z = relu(hh @ g("ge_fc_W").T + g("ge_fc_b"))
    hh = relu(z @ g("gd_fc_W").T + g("gd_fc_b"))
    hh = relu(_gat_np(hh, src, dst, mult, g("gd1_W"),
                      g("gd1_asrc"), g("gd1_adst"), g("gd1_b")))
    y = relu(_gat_np(hh, src, dst, mult, g("gd2_W"),
                     g("gd2_asrc"), g("gd2_adst"), g("gd2_b")))  # [96, H]
    yd = y.reshape(2, T, H)
    o2 = _lstm2_np(
        yd, g("td_Wih0"), g("td_Whh0"), g("td_bih0"), g("td_bhh0"),
        g("td_Wih1"), g("td_Whh1"), g("td_bih1"), g("td_bhh1"),
    )
    out[0:2, 0] = (1 / (1 + np.exp(-o2))).astype(np.float32)


# ---------------------------------------------------------------- kernel ---


def kernel(**inputs):
    from concourse.bass_utils import run_bass_kernel_spmd

    p = {k: np.asarray(v) for k, v in inputs.items()}
    if "nc" not in _CACHE:
        _CACHE["nc"] = _build_module(T)
    nc = _CACHE["nc"]

    w = _prep_weights(p)
    x = np.asarray(p["x"], np.float32)  # [N, B, T, F]
    xt = np.ascontiguousarray(x.transpose(3, 2, 0, 1))  # [F, T, N, B]
    embr = np.ascontiguousarray(
        np.vstack(
            [
                np.repeat(
                    np.asarray(p["emb"], np.float32).T[:, :, None], BPC, axis=2
                ).reshape(EMB, R),
                np.ones((1, R), np.float32),
            ]
        )
    )
    in_maps = []
    for i in range(NCORES):
        m = dict(w)
        m["embr"] = embr
        m["xin"] = np.ascontiguousarray(
            xt[:, :, :, i * BPC : (i + 1) * BPC].reshape(F, T, R)
        )
        in_maps.append(m)

    res = run_bass_kernel_spmd(nc, in_maps, core_ids=list(range(NCORES)))

    out = np.empty((N, B, T, F), np.float32)
    for i in range(NCORES):
        yo = res.results[i]["yout"]  # [128, T*40]
        rows = (
            yo.reshape(128, T, 4, F).transpose(2, 0, 1, 3).reshape(R, T, F)
        )  # r = k*128+p = n*8+b'
        out[:, i * BPC : (i + 1) * BPC] = rows.reshape(N, BPC, T, F)

    _host_correction(p, out)
    return out


# revision 48
# speedup vs baseline: 2.1510x; 1.0064x over previous
"""AttentionSTAE on 8 Trainium2 cores (Bass/Tile).

Structure (hardcoded shapes: N=64, B=64, T=48, F=10, EMB=16, H=128, E=256):

  Sharded data-parallel over batch B: core i handles batch slice
  b in [8i, 8i+8) -> 512 LSTM rows (r = n*8 + b'), pipelined over t:

    enc LSTM L0 (feature-major, gates [i|f|o|g] in PSUM)
    -> enc LSTM L1 (lagged one step)
    -> 6-layer dense per-node MLP (the GAT collapses to a dense layer for
       every node except global graph rows q<64; see host correction below)
    -> dec LSTM L0/L1 (row-major, lagged) -> sigmoid -> out

  The reference tiles the same [2,256] edge list (ids < 64) B*T times
  without offsets, so only global rows q<64 (batch b=0 -> core 0) receive
  real messages; every other node's GAT output is x @ W.T + b. The exact
  GAT path for the 96 affected rows (q<96 covers every decoder row that
  sees a corrected value) is recomputed on the host in float64 and the two
  affected output rows (n=0,1 at b=0) are overwritten.
"""

import numpy as np

N, B, T, F, EMB, H, E = 64, 64, 48, 10, 16, 128, 256
NCORES = 8
BPC = B // NCORES  # batch per core
R = N * BPC  # 512 LSTM rows per core
NEG = np.float32(0.2)

_CACHE = {}


# ---------------------------------------------------------------- device ---


def _build_module(nsteps=T):
    import concourse.bacc as bacc
    import concourse.tile as tile
    from concourse import mybir

    F32 = mybir.dt.float32
    F32R = mybir.dt.float32r
    SIG = mybir.ActivationFunctionType.Sigmoid
    TANH = mybir.ActivationFunctionType.Tanh
    ADD = mybir.AluOpType.add
    MAX = mybir.AluOpType.max
    import concourse.bass as bass

    nc = bacc.Bacc("TRN2", target_bir_lowering=False, debug=False)

    def din(name, shape):
        return nc.dram_tensor(name, shape, F32, kind="ExternalInput")

    xin = din("xin", [F, nsteps, R])
    embr = din("embr", [EMB + 1, R])  # emb rows + ones row (bias trick)
    w0h = din("w0h", [H, 4 * H])
    w0xe = din("w0xe", [F + EMB + 1, 4 * H])
    w1x = din("w1x", [H, 4 * H])
    w1h = din("w1h", [H, 4 * H])
    b1c = din("b1c", [H, 4])
    mlp_dims = [(H, H), (H, 64), (64, 32), (32, 64), (64, H), (H, H)]
    mws = [din(f"mw{i}", [k, m]) for i, (k, m) in enumerate(mlp_dims)]
    mbs = [din(f"mb{i}", [m, 1]) for i, (k, m) in enumerate(mlp_dims)]
    wd0x = din("wd0x", [H, 40])
    wd0h_blk = din("wd0h_blk", [40, 160])
    wd1xh_blk = din("wd1xh_blk", [80, 160])
    bd0t = din("bd0t", [10, 160])
    bd1t = din("bd1t", [10, 160])
    ones10 = din("ones10", [10, 128])
    ident = din("ident", [H, H])
    zeros = din("zeros", [H, R])
    yout = nc.dram_tensor("yout", [128, nsteps * 40], F32, kind="ExternalOutput")

    def cap(base, col_off, dims, part=None):
        """Custom AP over a tile: base partition dim + free dims (elem units)."""
        b = base[:] if not isinstance(base, bass.AP) else base
        pdim = [list(b.ap[0])]
        if part is not None:
            pdim = [[b.ap[0][0], part]]
        return bass.AP(b.tensor, b.offset + col_off, pdim + [list(d) for d in dims])

    with tile.TileContext(nc) as tc:
        with (
            tc.tile_pool(name="pers", bufs=1) as pers,
            tc.tile_pool(name="s0p", bufs=3) as s0p,
            tc.tile_pool(name="s1p", bufs=3) as s1p,
            tc.tile_pool(name="tcp", bufs=3) as tcp,
            tc.tile_pool(name="uvp", bufs=6) as uvp,
            tc.tile_pool(name="mlpa", bufs=3) as mlpa,
            tc.tile_pool(name="decp", bufs=2) as decp,
            tc.tile_pool(name="pIF", bufs=2, space="PSUM") as pIF,
            tc.tile_pool(name="pOG", bufs=1, space="PSUM") as pOG,
            tc.tile_pool(name="pMLP", bufs=1, space="PSUM") as pMLP,
            tc.tile_pool(name="pDEC", bufs=1, space="PSUM") as pDEC,
        ):
            # ---- persistent tiles ----
            h0 = pers.tile([H, R], F32R)
            h1 = pers.tile([H, R], F32R)
            c01 = pers.tile([H, 2 * R], F32)
            cd = pers.tile([128, 80], F32)
            outbuf = pers.tile([128, 80 + nsteps * 40], F32R)
            hd_stage = pers.tile([128, 80], F32R)  # [hd0(t1-1) | hd1(t2-1)]
            hdT = pers.tile([80, 128], F32R)
            xeA = pers.tile([F + EMB + 1, R], F32R)
            xeB = pers.tile([F + EMB + 1, R], F32R)
            w0h_s = pers.tile([H, 4 * H], F32R)
            w0xe_s = pers.tile([F + EMB + 1, 4 * H], F32R)
            w1x_s = pers.tile([H, 4 * H], F32R)
            w1h_s = pers.tile([H, 4 * H], F32R)
            b1_s = pers.tile([H, 4], F32)
            mw_s = [
                pers.tile([k, m], F32R, name=f"mws{i}")
                for i, (k, m) in enumerate(mlp_dims)
            ]
            mb_s = [
                pers.tile([m, 1], F32, name=f"mbs{i}")
                for i, (k, m) in enumerate(mlp_dims)
            ]
            wd0x_s = pers.tile([H, 40], F32R)
            wd0h_s = pers.tile([40, 160], F32R)
            wd1xh_s = pers.tile([80, 160], F32R)
            bd0t_s = pers.tile([10, 160], F32R)
            bd1t_s = pers.tile([10, 160], F32R)
            ones10_s = pers.tile([10, 128], F32R)
            ident_s = pers.tile([H, H], F32R)

            r32 = lambda ap: ap.bitcast(F32R)
            for dst, src in [
                (w0h_s, w0h), (w0xe_s, w0xe), (w1x_s, w1x), (w1h_s, w1h),
                (wd0x_s, wd0x), (wd0h_s, wd0h_blk), (wd1xh_s, wd1xh_blk),
                (bd0t_s, bd0t), (bd1t_s, bd1t), (ones10_s, ones10),
                (ident_s, ident),
            ]:
                nc.sync.dma_start(dst[:], r32(src[:]))
            nc.sync.dma_start(b1_s[:], b1c[:])
            for i in range(6):
                nc.sync.dma_start(mw_s[i][:], r32(mws[i][:]))
                nc.sync.dma_start(mb_s[i][:], mbs[i][:])
            nc.sync.dma_start(xeA[F : F + EMB + 1, :], r32(embr[:]))
            nc.sync.dma_start(xeB[F : F + EMB + 1, :], r32(embr[:]))

            nc.sync.dma_start(h0[:], r32(zeros[:]))
            nc.sync.dma_start(h1[:], r32(zeros[:]))
            nc.sync.dma_start(hd_stage[:], r32(zeros[:, 0:80]))
            nc.gpsimd.memset(c01[:], 0.0)
            nc.gpsimd.memset(cd[:], 0.0)


            KS = nsteps + 2  # python pipeline iters
            for k in range(KS):
                has_l0 = k < nsteps
                has_l1 = 1 <= k <= nsteps
                has_dec1 = k >= 2
                t0 = k  # enc L0 step
                t1 = k - 1  # enc L1 / MLP / dec L0 step
                t2 = k - 2  # dec L1 step

                # ---------------- encoder layer 0 (t0) ----------------
                if has_l0:
                    xe = xeA if k % 2 == 0 else xeB
                    nc.sync.dma_start(
                        xe[0:F, :], r32(xin[:, t0, :])
                    )
                    p0if = pIF.tile([H, 2 * R], F32, tag="if", name="p0if")
                    p0og = pOG.tile([H, 2 * R], F32, tag="og", name="p0og")
                    p0o = p0og[:, 0:R]
                    p0g = p0og[:, R : 2 * R]
                    for m in range(4):
                        reg = (
                            p0if[:, (m % 2) * R : (m % 2) * R + R]
                            if m < 2
                            else (p0o if m == 2 else p0g)
                        )
                        nc.tensor.matmul(
                            reg, w0h_s[:, m * H : (m + 1) * H], h0[:],
                            start=True, stop=False,
                        )
                        nc.tensor.matmul(
                            reg, w0xe_s[:, m * H : (m + 1) * H], xe[:],
                            start=False, stop=True,
                        )
                    s0 = s0p.tile([H, 4 * R], F32, tag="s0")
                    nc.scalar.activation(s0[:, 0 : 2 * R], p0if[:], SIG)
                    nc.scalar.activation(s0[:, 3 * R : 4 * R], p0g, TANH)
                    nc.scalar.activation(s0[:, 2 * R : 3 * R], p0o, SIG)
                    u0 = uvp.tile([H, R], F32, tag="uv")
                    v0 = uvp.tile([H, R], F32, tag="uv")
                    nc.gpsimd.tensor_mul(v0[:], s0[:, R : 2 * R], c01[:, 0:R])
                    nc.vector.tensor_mul(u0[:], s0[:, 0:R], s0[:, 3 * R : 4 * R])
                    nc.vector.tensor_add(c01[:, 0:R], u0[:], v0[:])

                # ---------------- encoder layer 1 (t1) ----------------
                if has_l1:
                    p1if = pIF.tile([H, 2 * R], F32, tag="if", name="p1if")
                    p1og = pOG.tile([H, 2 * R], F32, tag="og", name="p1og")
                    p1o = p1og[:, 0:R]
                    p1g = p1og[:, R : 2 * R]
                    for m in range(4):
                        reg = (
                            p1if[:, (m % 2) * R : (m % 2) * R + R]
                            if m < 2
                            else (p1o if m == 2 else p1g)
                        )
                        nc.tensor.matmul(
                            reg, w1h_s[:, m * H : (m + 1) * H], h1[:],
                            start=True, stop=False,
                        )
                        nc.tensor.matmul(
                            reg, w1x_s[:, m * H : (m + 1) * H], h0[:],
                            start=False, stop=True,
                        )
                    s1 = s1p.tile([H, 4 * R], F32, tag="s1")
                    nc.scalar.activation(s1[:, R : 2 * R], p1if[:, R : 2 * R], SIG, bias=b1_s[:, 1:2])
                    nc.scalar.activation(s1[:, 0:R], p1if[:, 0:R], SIG, bias=b1_s[:, 0:1])
                    nc.scalar.activation(s1[:, 3 * R : 4 * R], p1g, TANH, bias=b1_s[:, 3:4])
                    nc.scalar.activation(s1[:, 2 * R : 3 * R], p1o, SIG, bias=b1_s[:, 2:3])
                    u1 = uvp.tile([H, R], F32, tag="uv")
                    v1 = uvp.tile([H, R], F32, tag="uv")
                    nc.gpsimd.tensor_mul(v1[:], s1[:, R : 2 * R], c01[:, R : 2 * R])
                    nc.vector.tensor_mul(u1[:], s1[:, 0:R], s1[:, 3 * R : 4 * R])
                    nc.vector.tensor_add(c01[:, R : 2 * R], u1[:], v1[:])

                # tanh(c) split per layer: keeps the L0 recurrence loop short
                tc01 = tcp.tile([H, 2 * R], F32, tag="tc")
                if has_l0:
                    nc.scalar.activation(tc01[:, 0:R], c01[:, 0:R], TANH)
                    nc.vector.tensor_mul(h0[:], s0[:, 2 * R : 3 * R], tc01[:, 0:R])
                if has_l1:
                    nc.scalar.activation(tc01[:, R : 2 * R], c01[:, R : 2 * R], TANH)
                    nc.vector.tensor_mul(h1[:], s1[:, 2 * R : 3 * R], tc01[:, R : 2 * R])

                # ---------------- MLP (t1) ----------------
                if has_l1:
                    pm = pMLP.tile([H, R], F32, tag="mlp", name="pm")
                    prev = None
                    for i, (kk, mm) in enumerate(mlp_dims):
                        rhs = h1[:] if i == 0 else prev[0:kk, :]
                        reg = pm[0:mm, :]
                        nc.tensor.matmul(reg, mw_s[i][:], rhs, start=True, stop=True)
                        a = mlpa.tile([H, R], F32R, tag=f"a{i % 2}")
                        nc.vector.tensor_scalar(
                            a[0:mm, :], reg, mb_s[i][:, 0:1], 0.0, ADD, MAX
                        )
                        prev = a
                    y_t = prev  # [128, R] f32r tile

                # ---------------- decoder (dec L0 @ t1, dec L1 @ t2) ----------
                if has_l1 or has_dec1:
                    pd = pDEC.tile([H, R], F32, tag="dec", name="pd")
                    # transpose [hd0(t1-1) | hd1(t2-1)] -> hdT
                    if k >= 3:
                        nc.vector.tensor_copy(
                            hd_stage[:, 40:80],
                            outbuf[:, 80 + (k - 3) * 40 : 120 + (k - 3) * 40],
                        )
                    tps = cap(pd, 320, [[1, 128]], part=80).bitcast(F32R)
                    nc.tensor.transpose(tps, hd_stage[:], ident_s[:])
                    nc.vector.tensor_copy(hdT[:], tps)

                if has_l1:  # dec L0 at t1
                    nc.tensor.matmul(
                        pd[:, 0:160], ones10_s[:], bd0t_s[:],
                        start=True, stop=False, skip_group_check=True,
                    )
                    for kk in range(4):
                        nc.tensor.matmul(
                            pd[:, kk * 40 : kk * 40 + 40],
                            y_t[:, kk * 128 : (kk + 1) * 128],
                            wd0x_s[:], start=False, stop=False,
                            skip_group_check=True,
                        )
                    nc.tensor.matmul(
                        pd[:, 0:160], hdT[0:40, :], wd0h_s[:],
                        start=False, stop=True, skip_group_check=True,
                    )
                if has_dec1:  # dec L1 at t2 (x from hd0, h from hd1: one mm)
                    nc.tensor.matmul(
                        pd[:, 160:320], hdT[:], wd1xh_s[:],
                        start=True, stop=False, skip_group_check=True,
                    )
                    nc.tensor.matmul(
                        pd[:, 160:320], ones10_s[:], bd1t_s[:],
                        start=False, stop=True, skip_group_check=True,
                    )

                if has_l1 or has_dec1:
                    lt_off = 0 if has_l1 else 4  # active lt range
                    lt_cnt = (4 if has_l1 else 0) + (4 if has_dec1 else 0)
                    sgd = decp.tile([128, 240], F32, tag="sgd")
                    tgd = decp.tile([128, 80], F32, tag="tgd")
                    po, so = lt_off * 40, lt_off * 30
                    nc.scalar.activation(
                        cap(sgd, so, [[30, lt_cnt], [1, 30]]),
                        cap(pd, po, [[40, lt_cnt], [1, 30]]), SIG,
                    )
                    nc.scalar.activation(
                        cap(tgd, lt_off * 10, [[10, lt_cnt], [1, 10]]),
                        cap(pd, po + 30, [[40, lt_cnt], [1, 10]]), TANH,
                    )
                    ud = decp.tile([128, 80], F32, tag="ud")
                    vd = decp.tile([128, 80], F32, tag="vd")
                    co = lt_off * 10
                    cw = lt_cnt * 10
                    i_ap = cap(sgd, so, [[30, lt_cnt], [1, 10]])
                    f_ap = cap(sgd, so + 10, [[30, lt_cnt], [1, 10]])
                    o_ap = cap(sgd, so + 20, [[30, lt_cnt], [1, 10]])
                    g_ap = cap(tgd, co, [[10, lt_cnt], [1, 10]])
                    nc.vector.tensor_mul(ud[:, co : co + cw], i_ap, g_ap)
                    nc.vector.tensor_mul(vd[:, co : co + cw], f_ap, cd[:, co : co + cw])
                    nc.vector.tensor_add(
                        cd[:, co : co + cw], ud[:, co : co + cw], vd[:, co : co + cw]
                    )
                    tcd = decp.tile([128, 80], F32, tag="tcd")
                    nc.scalar.activation(tcd[:, co : co + cw], cd[:, co : co + cw], TANH)
                    if has_l1:
                        nc.vector.tensor_mul(
                            hd_stage[:, 0:40],
                            cap(sgd, 20, [[30, 4], [1, 10]]),
                            tcd[:, 0:40],
                        )
                    if has_dec1:
                        nc.vector.tensor_mul(
                            outbuf[:, 80 + t2 * 40 : 120 + t2 * 40],
                            cap(sgd, 140, [[30, 4], [1, 10]]),
                            tcd[:, 40:80],
                        )

            # ---- final sigmoid + writeback ----
            sig_out = pers.tile([128, nsteps * 40], F32)
            nc.scalar.activation(sig_out[:], outbuf[:, 80 : 80 + nsteps * 40], SIG)
            nc.sync.dma_start(yout[:], sig_out[:])

    nc.compile()
    return nc


# ------------------------------------------------------------------ host ---


def _perm(w, h):
    """reorder torch gate blocks [i,f,g,o] -> [i,f,o,g] along axis 0"""
    idx = np.r_[0:h, h : 2 * h, 3 * h : 4 * h, 2 * h : 3 * h]
    return w[idx]


def _prep_weights(p):
    f32 = np.float32
    out = {}
    b0 = _perm(p["te_bih0"] + p["te_bhh0"], H)
    out["w0h"] = np.ascontiguousarray(_perm(p["te_Whh0"], H).T, f32)
    out["w0xe"] = np.ascontiguousarray(
        np.vstack([_perm(p["te_Wih0"], H).T, b0[None, :]]), f32
    )
    out["w1x"] = np.ascontiguousarray(_perm(p["te_Wih1"], H).T, f32)
    out["w1h"] = np.ascontiguousarray(_perm(p["te_Whh1"], H).T, f32)
    b1 = _perm(p["te_bih1"] + p["te_bhh1"], H)
    out["b1c"] = np.ascontiguousarray(b1.reshape(4, H).T, f32)
    mlp = [
        ("ge1_W", "ge1_b"), ("ge2_W", "ge2_b"), ("ge_fc_W", "ge_fc_b"),
        ("gd_fc_W", "gd_fc_b"), ("gd1_W", "gd1_b"), ("gd2_W", "gd2_b"),
    ]
    for i, (wn, bn) in enumerate(mlp):
        out[f"mw{i}"] = np.ascontiguousarray(p[wn].T, f32)
        out[f"mb{i}"] = np.ascontiguousarray(p[bn][:, None], f32)
    bd0 = _perm(p["td_bih0"] + p["td_bhh0"], F)
    bd1 = _perm(p["td_bih1"] + p["td_bhh1"], F)
    def blkdiag(w):  # [10, 40] -> [40, 160] block-diagonal over 4 row tiles
        z = np.zeros((40, 160), f32)
        for k in range(4):
            z[k * 10 : (k + 1) * 10, k * 40 : (k + 1) * 40] = w
        return z

    wd0h = _perm(p["td_Whh0"], F).T
    wd1x = _perm(p["td_Wih1"], F).T
    wd1h = _perm(p["td_Whh1"], F).T
    out["wd0x"] = np.ascontiguousarray(_perm(p["td_Wih0"], F).T, f32)
    out["wd0h_blk"] = blkdiag(wd0h)
    out["wd1xh_blk"] = np.vstack([blkdiag(wd1x), blkdiag(wd1h)])
    out["bd0t"] = np.ascontiguousarray(
        np.tile(np.tile(bd0 / 10.0, 4)[None, :], (10, 1)), f32
    )
    out["bd1t"] = np.ascontiguousarray(
        np.tile(np.tile(bd1 / 10.0, 4)[None, :], (10, 1)), f32
    )
    out["ones10"] = np.ones((10, 128), f32)
    out["ident"] = np.eye(128, dtype=f32)
    out["zeros"] = np.zeros((H, R), f32)
    return out


def _lstm2_np(x, Wih0, Whh0, bih0, bhh0, Wih1, Whh1, bih1, bhh1):
    def layer(inp, Wih, Whh, bih, bhh):
        Rr, Tt, _ = inp.shape
        Hh = Whh.shape[1]
        xW = inp @ Wih.T + (bih + bhh)
        h = np.zeros((Rr, Hh))
        c = np.zeros((Rr, Hh))
        outs = np.empty((Rr, Tt, Hh))
        for t in range(Tt):
            g = xW[:, t] + h @ Whh.T
            i = 1 / (1 + np.exp(-g[:, :Hh]))
            f = 1 / (1 + np.exp(-g[:, Hh : 2 * Hh]))
            gg = np.tanh(g[:, 2 * Hh : 3 * Hh])
            o = 1 / (1 + np.exp(-g[:, 3 * Hh :]))
            c = f * c + i * gg
            h = o * np.tanh(c)
            outs[:, t] = h
        return outs

    return layer(layer(x, Wih0, Whh0, bih0, bhh0), Wih1, Whh1, bih1, bhh1)


def _gat_np(x, src, dst, mult, W, a_s, a_d, b):
    heads, od = a_s.shape
    h = (x @ W.T).reshape(-1, heads, od)
    es = (h * a_s[None]).sum(-1)
    ed = (h * a_d[None]).sum(-1)
    e_self = es + ed
    e_self = np.where(e_self >= 0, e_self, 0.2 * e_self)
    eb = es[src] + ed[dst]
    eb = np.where(eb >= 0, eb, 0.2 * eb)
    m = e_self.copy()
    np.maximum.at(m, dst, eb)
    ex_self = np.exp(e_self - m)
    ex_b = np.exp(eb - m[dst]) * float(mult)
    den = ex_self.copy()
    np.add.at(den, dst, ex_b)
    num = ex_self[..., None] * h
    np.add.at(num, dst, ex_b[..., None] * h[src])
    return (num / den[..., None]).reshape(-1, heads * od) + b


def _host_correction(p, out):
    """Recompute out[n=0:2, b=0] exactly (f64) with the true GAT messages."""
    f64 = np.float64
    g = lambda k: np.asarray(p[k], f64)
    x2 = g("x")[0:2, 0]  # [2, T, F]
    emb2 = g("emb")[0:2]
    hin = np.concatenate(
        [x2, np.broadcast_to(emb2[:, None, :], (2, T, EMB))], -1
    )  # [2, T, 26]
    th2 = _lstm2_np(
        hin, g("te_Wih0"), g("te_Whh0"), g("te_bih0"), g("te_bhh0"),
        g("te_Wih1"), g("te_Whh1"), g("te_bih1"), g("te_bhh1"),
    )  # [2, T, H]
    xf = th2.reshape(2 * T, H)  # rows q = n*T + t, q < 96
    src = np.asarray(p["distance_adj"])[0].astype(np.int64)
    dst = np.asarray(p["distance_adj"])[1].astype(np.int64)
    relu = lambda v: np.maximum(v, 0.0)
    mult = B * T
    hh = relu(_gat_np(xf, src, dst, mult, g("ge1_W"),
                      g("ge1_asrc"), g("ge1_adst"), g("ge1_b")))
    hh = relu(_gat_np(hh, src, dst, mult, g("ge2_W"),
                      g("ge2_asrc"), g("ge2_adst"), g("ge2_b")))
    z = relu(hh @ g("ge_fc_W").T + g("ge_fc_b"))
    hh = relu(z @ g("gd_fc_W").T + g("gd_fc_b"))
    hh = relu(_gat_np(hh, src, dst, mult, g("gd1_W"),
                      g("gd1_asrc"), g("gd1_adst"), g("gd1_b")))
    y = relu(_gat_np(hh, src, dst, mult, g("gd2_W"),
                     g("gd2_asrc"), g("gd2_adst"), g("gd2_b")))  # [96, H]
    yd = y.reshape(2, T, H)
    o2 = _lstm2_np(
        yd, g("td_Wih0"), g("td_Whh0"), g("td_bih0"), g("td_bhh0"),
        g("td_Wih1"), g("td_Whh1"), g("td_bih1"), g("td_bhh1"),
    )
    out[0:2, 0] = (1 / (1 + np.exp(-o2))).astype(np.float32)


# ---------------------------------------------------------------- kernel ---


def kernel(**inputs):
    from concourse.bass_utils import run_bass_kernel_spmd

    p = {k: np.asarray(v) for k, v in inputs.items()}
    if "nc" not in _CACHE:
        _CACHE["nc"] = _build_module(T)
    nc = _CACHE["nc"]

    w = _prep_weights(p)
    x = np.asarray(p["x"], np.float32)  # [N, B, T, F]
    xt = np.ascontiguousarray(x.transpose(3, 2, 0, 1))  # [F, T, N, B]
    embr = np.ascontiguousarray(
        np.vstack(
            [
                np.repeat(
                    np.asarray(p["emb"], np.float32).T[:, :, None], BPC, axis=2
                ).reshape(EMB, R),
                np.ones((1, R), np.float32),
            ]
        )
    )
    in_maps = []
    for i in range(NCORES):
        m = dict(w)
        m["embr"] = embr
        m["xin"] = np.ascontiguousarray(
            xt[:, :, :, i * BPC : (i + 1) * BPC].reshape(F, T, R)
        )
        in_maps.append(m)

    res = run_bass_kernel_spmd(nc, in_maps, core_ids=list(range(NCORES)))

    out = np.empty((N, B, T, F), np.float32)
    for i in range(NCORES):
        yo = res.results[i]["yout"]  # [128, T*40]
        rows = (
            yo.reshape(128, T, 4, F).transpose(2, 0, 1, 3).reshape(R, T, F)
        )  # r = k*128+p = n*8+b'
        out[:, i * BPC : (i + 1) * BPC] = rows.reshape(N, BPC, T, F)

    _host_correction(p, out)
    return out
